# revision 14
# baseline (speedup 1.0000x reference)
"""AdaptiveINR Trainium2 kernel (8-core data parallel).

Pipeline per image (one image per NeuronCore):
  contrast  = 0.5*sobel + 0.3*|lapl| + 0.2*gradm    (3x3 stencils on gray)
  density   = 0.1 + 0.9*sqrt(gauss5x5(minmax_norm(contrast)))
  coords    = base + (d>0.4)*off ;  weights = where(s<ns, d/ns, 0)
where d = density sampled at even rows/cols, off = input-independent
jax.random offsets (key 42) precomputed on host CPU.

Layout: 9 overlapping row-tiles per image, tile w holds image rows
122*w-3 .. 122*w+124 in partitions 0..127 (out-of-range rows zeroed).
Vertical convs (and the vertical half of the separable gauss, with the
horizontal taps accumulated in PSUM) are banded matmuls on TensorE;
horizontal shifts are shifted-AP ops on VectorE/GpSimd; transcendentals
on ScalarE.
"""

import functools
import os
import sys
import types

import numpy as np

B, H, W = 8, 1024, 1024
TH, TW = 512, 512
NT = 9          # row tiles per image
STRIDE = 122    # valid rows per tile
PAD = 3         # halo rows above the valid range

WR, WG, WB = 0.299, 0.587, 0.114
MIN_D, MAX_D = 0.1, 1.0
SPAN = MAX_D - MIN_D  # 0.9
LARGE = 1e30

TRACE = bool(os.environ.get("BASS_KERNEL_TRACE"))
LAST_EXEC_TIME_NS = None
LAST_RESULT = None


def _ensure_axon_hooks():
    """Register the NTFF profile hook (missing antenv.axon_hooks on this image)."""
    if "antenv.axon_hooks" in sys.modules:
        return
    try:
        import antenv  # noqa: F401

        mod = types.ModuleType("antenv.axon_hooks")
        _hook = {}
        mod.set_axon_ntff_profile_hook = lambda h: _hook.__setitem__("h", h)
        mod.get_axon_ntff_profile_hook = lambda: _hook.get("h")
        sys.modules["antenv.axon_hooks"] = mod
        from trn_agent_boot.trn_boot import _ntff_profile_via_ctypes

        mod.set_axon_ntff_profile_hook(
            _ntff_profile_via_ctypes("/opt/axon/libaxon_pjrt.so")
        )
    except Exception:
        pass


def _gauss1d():
    sigma = 5 / 6.0
    r = np.arange(5, dtype=np.float32) - 2
    k1 = np.exp(-0.5 * r**2 / sigma**2)
    k1 = (k1 / k1.sum()).astype(np.float32)
    return k1


@functools.lru_cache(maxsize=1)
def _host_consts():
    import jax
    import jax.numpy as jnp

    with jax.default_device(jax.devices("cpu")[0]):
        u = jax.random.uniform(jax.random.key(42), (B, TH, TW, 4, 2), jnp.float32)
        off = np.asarray((u - 0.5) * 0.8 * (2.0 / TH)).astype(np.float32)

    y = np.linspace(-1.0, 1.0, TH, dtype=np.float32)
    x = np.linspace(-1.0, 1.0, TW, dtype=np.float32)
    base = np.empty((TH, TW, 2), np.float32)
    base[:, :, 0] = y[:, None]
    base[:, :, 1] = x[None, :]
    off_all = np.ascontiguousarray(
        off.reshape(B, 128, 4, TW, 4, 2).reshape(B, 128, 4 * TW * 4 * 2))

    p0 = np.empty((128, 4, TW, 2), np.float32)
    p0[:, :, :, 0] = y.reshape(128, 4, 1)
    p0[:, :, :, 1] = x.reshape(1, 1, TW)
    p0 = np.ascontiguousarray(p0.reshape(128, 4 * TW * 2))

    # band matrices: band[k, m] = coeff of input partition k for output m
    def band(coeffs, zero_lo=0, zero_hi=0):
        bm = np.zeros((128, 128), np.float32)
        for dk, c in coeffs.items():
            for m in range(128):
                k = m + dk
                if 0 <= k < 128:
                    bm[k, m] = c
        if zero_lo:
            bm[:zero_lo, :] = 0.0
        if zero_hi:
            bm[-zero_hi:, :] = 0.0
        return bm

    k1 = _gauss1d()
    g0, g1, g2 = float(k1[0]), float(k1[1]), float(k1[2])
    band_v2 = band({-1: -1.0, 1: 1.0})
    band_2v2 = band({-1: -2.0, 1: 2.0})
    band_121 = band({-1: 1.0, 0: 2.0, 1: 1.0})
    band_l = band({-1: -1.0, 0: 4.0, 1: -1.0})
    gbands = []
    gb1 = {dk: float(k1[dk + 2]) * g1 for dk in range(-2, 3)}
    for scale in (g2 / g1, g0 / g1, 1.0):  # applied to nrm, h2, h1
        gb = {dk: v * scale for dk, v in gb1.items()}
        gbands.append([
            band(gb, zero_lo=PAD),           # tile 0
            band(gb),                        # middle
            band(gb, zero_hi=128 - 51),      # tile 8 (partitions >=51 are rows >=1024)
        ])

    # untrusted (non-duplicate, non-valid) rows for the min/max reduction
    maskl = np.zeros((128, NT), np.float32)
    maskl[0, :] = LARGE
    maskl[127, :] = LARGE
    maskl[1:3, 0] = LARGE
    maskl[51:, NT - 1] = LARGE

    return dict(
        off_all=off_all, p0=p0, maskl=np.ascontiguousarray(maskl),
        band_v2=band_v2, band_2v2=band_2v2, band_121=band_121, band_l=band_l,
        gbands=gbands, g0=g0, g1=g1, g2=g2,
    )


@functools.lru_cache(maxsize=1)
def _custom_ops():
    import concourse.dve_ops as dve_ops
    from concourse.dve_spec import Spec, Src0, Src1, C0, C1, C2, Zero, lower, sq, maxx
    from concourse.dve_uop import DveOpSpec

    def author(name, spec, rd1=True):
        if name in dve_ops._SUB_OPCODE_FOR_NAME:
            for op in dve_ops.OPS:
                if op.name == name:
                    return op
        row = max(dve_ops._SUB_OPCODE_FOR_NAME.values()) + 1
        dve_ops._SUB_OPCODE_FOR_NAME[name] = row
        shas = {}
        for ver in ("v3", "v4"):
            uops = lower(spec, ver=ver)
            shas[ver] = DveOpSpec(name=name, opcode=row, uops=uops, rd1_en=rd1).sha(ver)
        op = dve_ops.DveOp(name, spec, subdim=False, uops_sha=shas)
        dve_ops.OPS.append(op)
        dve_ops.CUSTOM_DVE_SPECS[name] = spec
        return op

    SQSUM = author("ANT_SQSUM", Spec(
        body=sq(Src0) + sq(Src1),
        reference=lambda i0, i1, s0, s1, c2: (
            i0.astype(np.float32) ** 2 + i1.astype(np.float32) ** 2
        ),
    ))
    body_abs = maxx(Src0 - Src1, Zero - (Src0 - Src1)) * C2
    ABSSUBS = author("ANT_ABSSUBS", Spec(
        body=body_abs,
        reference=lambda i0, i1, s0, s1, c2: (np.abs(
            i0.astype(np.float32) - i1.astype(np.float32)) * c2),
    ))
    ADD_ACCMAX = author("ANT_ADD_ACCMAX", Spec(
        body=Src0 + Src1, accum=maxx, accum_init=C0,
        reference=lambda i0, i1, s0, s1, c2: (
            (i0 + i1).astype(np.float32),
            np.maximum(s0, (i0 + i1).max(axis=-1, keepdims=True)).astype(np.float32),
        ),
    ))
    body_wq = Src0 - C2 * ((Src1 > C0) * Src0 + C2 * ((Src0 > C1) * Src0))
    WQ = author("ANT_WQ", Spec(
        body=body_wq,
        reference=lambda i0, i1, s0, s1, c2: (
            i0 - (i0 * c2) * (i0 > s0) - ((i1 * c2) * c2) * (i1 > s1)
        ).astype(np.float32),
    ))
    return dict(SQSUM=SQSUM, ABSSUBS=ABSSUBS, ADD_ACCMAX=ADD_ACCMAX, WQ=WQ)


@functools.lru_cache(maxsize=1)
def _build():
    import concourse.bass as bass
    import concourse.tile as tile
    from concourse import bacc, mybir
    from concourse.alu_op_type import AluOpType
    from concourse import bass_isa

    C = _host_consts()
    OPS = _custom_ops()
    f32 = mybir.dt.float32
    X = mybir.AxisListType.X
    AF = mybir.ActivationFunctionType

    nc = bacc.Bacc()

    img_e = nc.declare_dram_parameter("img", [3, H, W], f32, isOutput=False)
    off_e = nc.declare_dram_parameter("off", [128, 4 * TW * 4 * 2], f32, isOutput=False)
    p0_e = nc.declare_dram_parameter("p0", [128, 4 * TW * 2], f32, isOutput=False)
    maskl_e = nc.declare_dram_parameter("maskl", [128, NT], f32, isOutput=False)
    bands_in = {}
    for nm in ("band_v2", "band_2v2", "band_121", "band_l"):
        bands_in[nm] = nc.declare_dram_parameter(nm, [128, 128], f32, isOutput=False)
    gband_in = []
    for gi in range(3):
        row = []
        for vi in range(3):
            nm = f"bg_{gi}_{vi}"
            row.append(nc.declare_dram_parameter(nm, [128, 128], f32, isOutput=False))
        gband_in.append(row)

    dens_e = nc.declare_dram_parameter("density", [H, W], f32, isOutput=True)
    w0_e = nc.declare_dram_parameter("w0", [128, 4 * TW], f32, isOutput=True)
    w1_e = nc.declare_dram_parameter("w1", [128, 4 * TW], f32, isOutput=True)
    w2_e = nc.declare_dram_parameter("w2", [128, 4 * TW], f32, isOutput=True)
    cout_e = nc.declare_dram_parameter("coords", [128, 4 * TW * 4 * 2], f32, isOutput=True)

    dcols_d = nc.dram_tensor("dcols_scratch", [H, TW], f32)

    with tile.TileContext(nc) as tc:
        with tc.tile_pool(name="persist", bufs=1) as pp:
            # constants
            bt = {}
            for nm, e in bands_in.items():
                t = pp.tile([128, 128], f32, tag=nm)
                nc.sync.dma_start(t[:], e[:])
                bt[nm] = t
            gbt = []
            for gi in range(3):
                row = []
                for vi in range(3):
                    t = pp.tile([128, 128], f32, tag=f"bg_{gi}_{vi}")
                    nc.sync.dma_start(t[:], gband_in[gi][vi][:])
                    row.append(t)
                gbt.append(row)
            masklt = pp.tile([128, NT], f32, tag="masklt")
            nc.sync.dma_start(masklt[:], maskl_e[:])
            cbias = pp.tile([128, 4], f32, tag="cbias")
            nc.vector.memset(cbias[:, 0:1], 2.5e-9)
            nc.vector.memset(cbias[:, 1:2], 4.0e-10)
            nc.vector.memset(cbias[:, 2:3], MIN_D)

            maxstack = pp.tile([128, NT], f32, tag="maxstack")
            minstack = pp.tile([128, NT], f32, tag="minstack")

            shp = tc.tile_pool(name="shp", bufs=1)
            shpool = shp.__enter__()
            CONT = shpool.tile([128, NT, W], f32, tag="CONT")
            scratch = shpool.tile([128, 4096], f32, tag="scratch")

            # ---------------- phase I: gray -> contrast ---------------------
            with tc.tile_pool(name="ph1", bufs=2) as wp, \
                 tc.tile_pool(name="io", bufs=3) as iop, \
                 tc.tile_pool(name="ps1", bufs=1, space="PSUM") as psp:
                for w in range(NT):
                    row0 = STRIDE * w - PAD
                    lo, hi = max(row0, 0), min(row0 + 128, H)
                    plo, phi = lo - row0, hi - row0

                    rgb = iop.tile([128, 3, W], f32, tag="rgb")
                    if plo > 0 or phi < 128:
                        nc.vector.memset(rgb[:], 0.0)
                    nc.sync.dma_start(
                        rgb[plo:phi, :, :],
                        img_e[:, lo:hi, :].rearrange("c r j -> r c j"))
                    R, G, Bc = rgb[:, 0, :], rgb[:, 1, :], rgb[:, 2, :]

                    t0 = wp.tile([128, W], f32, tag="t0")
                    nc.vector.scalar_tensor_tensor(
                        t0[:], R, WR / WG, G,
                        op0=AluOpType.mult, op1=AluOpType.add)
                    g = wp.tile([128, W], f32, tag="g")
                    nc.vector.affine_then_add(g[:], Bc, t0[:], WB / WG, 0.0)

                    b = wp.tile([128, W], f32, tag="b")
                    nc.gpsimd.tensor_tensor(
                        b[:, 1:1023], g[:, 0:1022], g[:, 2:1024], op=AluOpType.add)
                    nc.scalar.copy(b[:, 0:1], g[:, 1:2])
                    nc.scalar.copy(b[:, 1023:1024], g[:, 1022:1023])

                    hd = wp.tile([128, W], f32, tag="hd")
                    nc.gpsimd.tensor_tensor(
                        hd[:, 1:1023], g[:, 2:1024], g[:, 0:1022],
                        op=AluOpType.subtract)
                    nc.scalar.copy(hd[:, 0:1], g[:, 1:2])
                    nc.scalar.mul(hd[:, 1023:1024], g[:, 1022:1023], -1.0)

                    v2_ps = psp.tile([128, W], f32, tag="v2_ps")
                    gy_ps = psp.tile([128, W], f32, tag="gy_ps")
                    gx_ps = psp.tile([128, W], f32, tag="gx_ps")
                    q1_ps = psp.tile([128, W], f32, tag="q1_ps")
                    for hh in (slice(0, 512), slice(512, 1024)):
                        nc.tensor.matmul(v2_ps[:, hh], bt["band_v2"][:], g[:, hh])
                        nc.tensor.matmul(gy_ps[:, hh], bt["band_2v2"][:], g[:, hh],
                                         start=True, stop=False)
                        nc.tensor.matmul(gy_ps[:, hh], bt["band_v2"][:], b[:, hh],
                                         start=False, stop=True)
                        nc.tensor.matmul(gx_ps[:, hh], bt["band_121"][:], hd[:, hh])
                        nc.tensor.matmul(q1_ps[:, hh], bt["band_l"][:], g[:, hh])

                    gxs = wp.tile([128, W], f32, tag="gxs")
                    nc.scalar.copy(gxs[:], gx_ps[:])

                    sob = wp.tile([128, W], f32, tag="sob")
                    nc.vector._custom_dve(OPS["SQSUM"], out=sob[:], in0=gxs[:], in1=gy_ps[:])
                    lapl = wp.tile([128, W], f32, tag="lapl")
                    nc.vector._custom_dve(
                        OPS["ABSSUBS"], out=lapl[:], in0=q1_ps[:], in1=b[:],
                        imm2=0.3 * WG)

                    grad = wp.tile([128, W], f32, tag="grad")
                    nc.vector._custom_dve(OPS["SQSUM"], out=grad[:], in0=hd[:], in1=v2_ps[:])
                    nc.vector._custom_dve(
                        OPS["SQSUM"], out=grad[:, 0:1], in0=hd[:, 1:2], in1=v2_ps[:, 0:1])
                    nc.vector._custom_dve(
                        OPS["SQSUM"], out=grad[:, 1023:1024], in0=hd[:, 1022:1023],
                        in1=v2_ps[:, 1023:1024])
                    if w == 0 or w == NT - 1:
                        p = PAD if w == 0 else (1023 - row0)
                        pa, pb = (p + 2, p) if w == 0 else (p, p - 2)
                        nc.gpsimd.dma_start(scratch[0:1, 0:1024], hd[p:p + 1, :])
                        nc.scalar.copy(scratch[0:1, 0:1], scratch[0:1, 1:2])
                        nc.scalar.copy(scratch[0:1, 1023:1024], scratch[0:1, 1022:1023])
                        nc.gpsimd.dma_start(scratch[0:1, 1024:2048], g[pa:pa + 1, :])
                        nc.gpsimd.dma_start(scratch[0:1, 3072:4096], g[pb:pb + 1, :])
                        nc.vector.tensor_tensor(
                            scratch[0:1, 1024:2048], scratch[0:1, 1024:2048],
                            scratch[0:1, 3072:4096], op=AluOpType.subtract)
                        nc.vector._custom_dve(
                            OPS["SQSUM"], out=scratch[0:1, 2048:3072],
                            in0=scratch[0:1, 0:1024], in1=scratch[0:1, 1024:2048])
                        nc.gpsimd.dma_start(grad[p:p + 1, :], scratch[0:1, 2048:3072])

                    sob_s = wp.tile([128, W], f32, tag="sob_s")
                    nc.scalar.activation(
                        sob_s[:], sob[:], AF.Sqrt,
                        bias=cbias[:, 0:1], scale=0.25 * WG * WG)
                    grad_s = wp.tile([128, W], f32, tag="grad_s")
                    nc.scalar.activation(
                        grad_s[:], grad[:], AF.Sqrt,
                        bias=cbias[:, 1:2], scale=0.04 * WG * WG)

                    t2 = wp.tile([128, W], f32, tag="t2")
                    nc.gpsimd.tensor_tensor(t2[:], sob_s[:], lapl[:], op=AluOpType.add)

                    nc.vector._custom_dve(
                        OPS["ADD_ACCMAX"], out=CONT[:, w, :], in0=t2[:], in1=grad_s[:],
                        s0=-LARGE, accum_out=maxstack[:, w:w + 1])
                    nc.vector.tensor_reduce(
                        minstack[:, w:w + 1], CONT[:, w, :], axis=X, op=AluOpType.min)

            # ---------------- min/max aggregation ---------------------------
            mm1 = pp.tile([128, NT], f32, tag="mm1")
            nc.vector.tensor_tensor(mm1[:], maxstack[:], masklt[:], op=AluOpType.subtract)
            mm2 = pp.tile([128, NT], f32, tag="mm2")
            nc.vector.tensor_tensor(mm2[:], minstack[:], masklt[:], op=AluOpType.add)
            mxr = pp.tile([128, 1], f32, tag="mxr")
            nc.vector.tensor_reduce(mxr[:], mm1[:], axis=X, op=AluOpType.max)
            mnr = pp.tile([128, 1], f32, tag="mnr")
            nc.vector.tensor_reduce(mnr[:], mm2[:], axis=X, op=AluOpType.min)
            negmn = pp.tile([128, 1], f32, tag="negmn")
            nc.vector.tensor_single_scalar(negmn[:], mnr[:], -1.0, op=AluOpType.mult)
            mxa = pp.tile([128, 1], f32, tag="mxa")
            nc.gpsimd.partition_all_reduce(mxa[:], mxr[:], channels=128,
                                           reduce_op=bass_isa.ReduceOp.max)
            nga = pp.tile([128, 1], f32, tag="nga")
            nc.gpsimd.partition_all_reduce(nga[:], negmn[:], channels=128,
                                           reduce_op=bass_isa.ReduceOp.max)
            rng = pp.tile([128, 1], f32, tag="rng")
            nc.vector.tensor_tensor(rng[:], mxa[:], nga[:], op=AluOpType.add)
            inv = pp.tile([128, 1], f32, tag="inv")
            nc.vector.reciprocal(inv[:], rng[:])
            mnv = pp.tile([128, 1], f32, tag="mnv")
            nc.vector.tensor_single_scalar(mnv[:], nga[:], -1.0, op=AluOpType.mult)

            # ---------------- phase II: normalize + gauss + density ---------
            with tc.tile_pool(name="ph2", bufs=2) as wp2, \
                 tc.tile_pool(name="ps2", bufs=2, space="PSUM") as ps2:
                for w in range(NT):
                    vi = 0 if w == 0 else (2 if w == NT - 1 else 1)
                    nrm = wp2.tile([128, W], f32, tag="nrm")
                    nc.vector.tensor_scalar(
                        nrm[:], CONT[:, w, :], mnv[:], inv[:],
                        op0=AluOpType.subtract, op1=AluOpType.mult)
                    h1 = wp2.tile([128, W], f32, tag="h1")
                    nc.gpsimd.tensor_tensor(
                        h1[:, 1:1023], nrm[:, 0:1022], nrm[:, 2:1024], op=AluOpType.add)
                    nc.scalar.copy(h1[:, 0:1], nrm[:, 1:2])
                    nc.scalar.copy(h1[:, 1023:1024], nrm[:, 1022:1023])
                    h2 = wp2.tile([128, W], f32, tag="h2")
                    nc.gpsimd.tensor_tensor(
                        h2[:, 2:1022], nrm[:, 0:1020], nrm[:, 4:1024], op=AluOpType.add)
                    nc.scalar.copy(h2[:, 0:2], nrm[:, 2:4])
                    nc.scalar.copy(h2[:, 1022:1024], nrm[:, 1020:1022])

                    x_ps = ps2.tile([128, W], f32, tag="x_ps")
                    for hh in (slice(0, 512), slice(512, 1024)):
                        nc.tensor.matmul(x_ps[:, hh], gbt[0][vi][:], nrm[:, hh],
                                         start=True, stop=False)
                        nc.tensor.matmul(x_ps[:, hh], gbt[1][vi][:], h2[:, hh],
                                         start=False, stop=False)
                        nc.tensor.matmul(x_ps[:, hh], gbt[2][vi][:], h1[:, hh],
                                         start=False, stop=True)

                    r = wp2.tile([128, W], f32, tag="r")
                    nc.scalar.activation(r[:], x_ps[:], AF.Relu, scale=SPAN * SPAN)
                    qq = wp2.tile([128, W], f32, tag="qq")
                    nc.scalar.activation(qq[:], r[:], AF.Sqrt)
                    dens = wp2.tile([128, W], f32, tag="dens")
                    nc.scalar.activation(dens[:], qq[:], AF.Identity, bias=cbias[:, 2:3])
                    dcl = wp2.tile([128, TW], f32, tag="dcl")
                    nc.scalar.copy(dcl[:], dens[:, 0:1024:2])
                    lo_v = STRIDE * w
                    hi_v = min(lo_v + STRIDE, H)
                    pv0, pv1 = PAD, PAD + (hi_v - lo_v)
                    eng = (nc.sync, nc.scalar, nc.gpsimd)[w % 3]
                    eng.dma_start(dens_e[lo_v:hi_v, :], dens[pv0:pv1, :])
                    eng2 = (nc.scalar, nc.gpsimd, nc.sync)[w % 3]
                    eng2.dma_start(dcols_d[lo_v:hi_v, :], dcl[pv0:pv1, :])

            shp.__exit__(None, None, None)

            # ---------------- sample stage: weights + coords ----------------
            with tc.tile_pool(name="samp", bufs=1) as sp, \
                 tc.tile_pool(name="cq", bufs=2) as cqp:
                D = sp.tile([128, 4, TW], f32, tag="D")
                nc.sync.dma_start(
                    D[:], dcols_d[:].rearrange("(q s) j -> q s j", s=8)[:, 0:8:2, :])
                Dv = D[:].rearrange("p s j -> p (s j)")
                m2 = sp.tile([128, 4 * TW], f32, tag="m2")
                nc.vector.tensor_single_scalar(m2[:], Dv, 0.4, op=AluOpType.is_gt)
                m4 = sp.tile([128, 4 * TW], f32, tag="m4")
                nc.vector.tensor_single_scalar(m4[:], Dv, 0.7, op=AluOpType.is_gt)
                wq = sp.tile([128, 4 * TW], f32, tag="wq")
                nc.vector._custom_dve(
                    OPS["WQ"], out=wq[:], in0=Dv, in1=Dv, s0=0.4, s1=0.7, imm2=0.5)
                w1t = sp.tile([128, 4 * TW], f32, tag="w1t")
                nc.vector.tensor_tensor(w1t[:], wq[:], m2[:], op=AluOpType.mult)
                w2t = sp.tile([128, 4 * TW], f32, tag="w2t")
                nc.vector.tensor_tensor(w2t[:], wq[:], m4[:], op=AluOpType.mult)
                nc.sync.dma_start(w0_e[:], wq[:])
                nc.scalar.dma_start(w1_e[:], w1t[:])
                nc.gpsimd.dma_start(w2_e[:], w2t[:])

                p0t = sp.tile([128, 4 * TW * 2], f32, tag="p0t")
                nc.sync.dma_start(p0t[:], p0_e[:])
                # full-image planes over (s, c): off/out strided by 8, m2/p0 dense
                offt = sp.tile([128, 4 * TW * 4 * 2], f32, tag="offt")
                nc.sync.dma_start(offt[:, 0:8192], off_e[:, 0:8192])
                nc.scalar.dma_start(offt[:, 8192:16384], off_e[:, 8192:16384])
                cqt = sp.tile([128, 4 * TW * 4 * 2], f32, tag="cqt")
                offv = offt[:].rearrange("p (n s c) -> p n s c", s=4, c=2)
                cqv = cqt[:].rearrange("p (n s c) -> p n s c", s=4, c=2)
                p0v = p0t[:].rearrange("p (n c) -> p n c", c=2)
                for s_ in range(4):
                    for c_ in range(2):
                        teng = nc.gpsimd if (s_ * 2 + c_) % 2 == 0 else nc.vector
                        teng.tensor_tensor(
                            cqv[:, :, s_, c_], offv[:, :, s_, c_], m2[:],
                            op=AluOpType.mult)
                        aeng = nc.vector if (s_ * 2 + c_) % 2 == 0 else nc.gpsimd
                        aeng.tensor_tensor(
                            cqv[:, :, s_, c_], cqv[:, :, s_, c_], p0v[:, :, c_],
                            op=AluOpType.add)
                for r_ in range(4):
                    eng = (nc.sync, nc.scalar, nc.gpsimd, nc.sync)[r_]
                    eng.dma_start(
                        cout_e[:, r_ * 4096:(r_ + 1) * 4096],
                        cqt[:, r_ * 4096:(r_ + 1) * 4096])

    nc.finalize()
    return nc


def kernel(img, target_height, target_width, **_kw):
    global LAST_EXEC_TIME_NS, LAST_RESULT
    th, tw = int(target_height), int(target_width)
    img = np.ascontiguousarray(np.asarray(img, dtype=np.float32))
    assert img.shape == (B, 3, H, W) and th == TH and tw == TW, (
        f"kernel hardcoded for img(8,3,1024,1024), th=tw=512; got {img.shape} {th} {tw}")

    if TRACE:
        _ensure_axon_hooks()

    C = _host_consts()
    nc = _build()

    from concourse.bass_utils import run_bass_kernel_spmd

    in_maps = []
    for i in range(B):
        m = {
            "img": img[i],
            "off": C["off_all"][i],
            "p0": C["p0"],
            "maskl": C["maskl"],
            "band_v2": C["band_v2"],
            "band_2v2": C["band_2v2"],
            "band_121": C["band_121"],
            "band_l": C["band_l"],
        }
        for gi in range(3):
            for vi in range(3):
                m[f"bg_{gi}_{vi}"] = C["gbands"][gi][vi]
        in_maps.append(m)
    res = run_bass_kernel_spmd(
        nc, in_maps, core_ids=list(range(B)), trace=TRACE)
    LAST_EXEC_TIME_NS = res.exec_time_ns
    LAST_RESULT = res

    coords = np.empty((B, TH * TW, 4, 2), np.float32)
    weights = np.empty((B, TH * TW, 4), np.float32)
    density = np.empty((B, 1, H, W), np.float32)
    for i in range(B):
        r = res.results[i]
        coords[i] = r["coords"].reshape(TH * TW, 4, 2)
        w0 = r["w0"].reshape(-1)
        w1 = r["w1"].reshape(-1)
        w2 = r["w2"].reshape(-1)
        weights[i] = np.stack([w0, w1, w2, w2], axis=-1)
        density[i, 0] = r["density"]
    return coords, weights, density


# revision 17
# speedup vs baseline: 1.0084x; 1.0084x over previous
"""AdaptiveINR Trainium2 kernel (8-core data parallel).

Pipeline per image (one image per NeuronCore):
  contrast  = 0.5*sobel + 0.3*|lapl| + 0.2*gradm    (3x3 stencils on gray)
  density   = 0.1 + 0.9*sqrt(gauss5x5(minmax_norm(contrast)))
  coords    = base + (d>0.4)*off ;  weights = where(s<ns, d/ns, 0)
where d = density sampled at even rows/cols, off = input-independent
jax.random offsets (key 42) precomputed on host CPU.

Layout: 9 overlapping row-tiles per image, tile w holds image rows
122*w-3 .. 122*w+124 in partitions 0..127 (out-of-range rows zeroed).
Vertical convs (and the vertical half of the separable gauss, with the
horizontal taps accumulated in PSUM) are banded matmuls on TensorE;
horizontal shifts are shifted-AP ops on VectorE/GpSimd; transcendentals
on ScalarE.
"""

import functools
import os
import sys
import types

import numpy as np

B, H, W = 8, 1024, 1024
TH, TW = 512, 512
NT = 9          # row tiles per image
STRIDE = 122    # valid rows per tile
PAD = 3         # halo rows above the valid range

WR, WG, WB = 0.299, 0.587, 0.114
MIN_D, MAX_D = 0.1, 1.0
SPAN = MAX_D - MIN_D  # 0.9
LARGE = 1e30

TRACE = bool(os.environ.get("BASS_KERNEL_TRACE"))
LAST_EXEC_TIME_NS = None
LAST_RESULT = None


def _ensure_axon_hooks():
    """Register the NTFF profile hook (missing antenv.axon_hooks on this image)."""
    if "antenv.axon_hooks" in sys.modules:
        return
    try:
        import antenv  # noqa: F401

        mod = types.ModuleType("antenv.axon_hooks")
        _hook = {}
        mod.set_axon_ntff_profile_hook = lambda h: _hook.__setitem__("h", h)
        mod.get_axon_ntff_profile_hook = lambda: _hook.get("h")
        sys.modules["antenv.axon_hooks"] = mod
        from trn_agent_boot.trn_boot import _ntff_profile_via_ctypes

        mod.set_axon_ntff_profile_hook(
            _ntff_profile_via_ctypes("/opt/axon/libaxon_pjrt.so")
        )
    except Exception:
        pass


def _gauss1d():
    sigma = 5 / 6.0
    r = np.arange(5, dtype=np.float32) - 2
    k1 = np.exp(-0.5 * r**2 / sigma**2)
    k1 = (k1 / k1.sum()).astype(np.float32)
    return k1


@functools.lru_cache(maxsize=1)
def _host_consts():
    import jax
    import jax.numpy as jnp

    with jax.default_device(jax.devices("cpu")[0]):
        u = jax.random.uniform(jax.random.key(42), (B, TH, TW, 4, 2), jnp.float32)
        off = np.asarray((u - 0.5) * 0.8 * (2.0 / TH)).astype(np.float32)

    y = np.linspace(-1.0, 1.0, TH, dtype=np.float32)
    x = np.linspace(-1.0, 1.0, TW, dtype=np.float32)
    base = np.empty((TH, TW, 2), np.float32)
    base[:, :, 0] = y[:, None]
    base[:, :, 1] = x[None, :]
    p1b = (base[None, :, :, None, :] + off).astype(np.float32)  # [B, TH, TW, 4, 2]
    p1_all = np.ascontiguousarray(
        p1b.reshape(B, 128, 4, TW, 4, 2).reshape(B, 128, 4 * TW * 4 * 2))

    p0 = np.empty((128, 4, TW, 2), np.float32)
    p0[:, :, :, 0] = y.reshape(128, 4, 1)
    p0[:, :, :, 1] = x.reshape(1, 1, TW)
    p0 = np.ascontiguousarray(p0.reshape(128, 4 * TW * 2))

    # band matrices: band[k, m] = coeff of input partition k for output m
    def band(coeffs, zero_lo=0, zero_hi=0):
        bm = np.zeros((128, 128), np.float32)
        for dk, c in coeffs.items():
            for m in range(128):
                k = m + dk
                if 0 <= k < 128:
                    bm[k, m] = c
        if zero_lo:
            bm[:zero_lo, :] = 0.0
        if zero_hi:
            bm[-zero_hi:, :] = 0.0
        return bm

    k1 = _gauss1d()
    g0, g1, g2 = float(k1[0]), float(k1[1]), float(k1[2])
    band_v2 = band({-1: -1.0, 1: 1.0})
    band_2v2 = band({-1: -2.0, 1: 2.0})
    band_121 = band({-1: 1.0, 0: 2.0, 1: 1.0})
    band_l = band({-1: -1.0, 0: 4.0, 1: -1.0})
    gbands = []
    gb1 = {dk: float(k1[dk + 2]) * g1 for dk in range(-2, 3)}
    for scale in (g2 / g1, g0 / g1, 1.0):  # applied to nrm, h2, h1
        gb = {dk: v * scale for dk, v in gb1.items()}
        gbands.append([
            band(gb, zero_lo=PAD),           # tile 0
            band(gb),                        # middle
            band(gb, zero_hi=128 - 51),      # tile 8 (partitions >=51 are rows >=1024)
        ])

    # untrusted (non-duplicate, non-valid) rows for the min/max reduction
    maskl = np.zeros((128, NT), np.float32)
    maskl[0, :] = LARGE
    maskl[127, :] = LARGE
    maskl[1:3, 0] = LARGE
    maskl[51:, NT - 1] = LARGE

    allbands = np.ascontiguousarray(np.stack(
        [band_v2, band_2v2, band_121, band_l]
        + [gbands[gi][vi] for gi in range(3) for vi in range(3)], axis=0))
    return dict(
        p1_all=p1_all, p0=p0, maskl=np.ascontiguousarray(maskl),
        allbands=allbands, g0=g0, g1=g1, g2=g2,
    )


@functools.lru_cache(maxsize=1)
def _custom_ops():
    import concourse.dve_ops as dve_ops
    from concourse.dve_spec import Spec, Src0, Src1, C0, C1, C2, Zero, lower, sq, maxx
    from concourse.dve_uop import DveOpSpec

    def author(name, spec, rd1=True):
        if name in dve_ops._SUB_OPCODE_FOR_NAME:
            for op in dve_ops.OPS:
                if op.name == name:
                    return op
        row = max(dve_ops._SUB_OPCODE_FOR_NAME.values()) + 1
        dve_ops._SUB_OPCODE_FOR_NAME[name] = row
        shas = {}
        for ver in ("v3", "v4"):
            uops = lower(spec, ver=ver)
            shas[ver] = DveOpSpec(name=name, opcode=row, uops=uops, rd1_en=rd1).sha(ver)
        op = dve_ops.DveOp(name, spec, subdim=False, uops_sha=shas)
        dve_ops.OPS.append(op)
        dve_ops.CUSTOM_DVE_SPECS[name] = spec
        return op

    SQSUM = author("ANT_SQSUM", Spec(
        body=sq(Src0) + sq(Src1),
        reference=lambda i0, i1, s0, s1, c2: (
            i0.astype(np.float32) ** 2 + i1.astype(np.float32) ** 2
        ),
    ))
    body_abs = maxx(Src0 - Src1, Zero - (Src0 - Src1)) * C2
    ABSSUBS = author("ANT_ABSSUBS", Spec(
        body=body_abs,
        reference=lambda i0, i1, s0, s1, c2: (np.abs(
            i0.astype(np.float32) - i1.astype(np.float32)) * c2),
    ))
    ADD_ACCMAX = author("ANT_ADD_ACCMAX", Spec(
        body=Src0 + Src1, accum=maxx, accum_init=C0,
        reference=lambda i0, i1, s0, s1, c2: (
            (i0 + i1).astype(np.float32),
            np.maximum(s0, (i0 + i1).max(axis=-1, keepdims=True)).astype(np.float32),
        ),
    ))
    body_wq = Src0 - C2 * ((Src1 > C0) * Src0 + C2 * ((Src0 > C1) * Src0))
    WQ = author("ANT_WQ", Spec(
        body=body_wq,
        reference=lambda i0, i1, s0, s1, c2: (
            i0 - (i0 * c2) * (i0 > s0) - ((i1 * c2) * c2) * (i1 > s1)
        ).astype(np.float32),
    ))
    return dict(SQSUM=SQSUM, ABSSUBS=ABSSUBS, ADD_ACCMAX=ADD_ACCMAX, WQ=WQ)


@functools.lru_cache(maxsize=1)
def _build():
    import concourse.bass as bass
    import concourse.tile as tile
    from concourse import bacc, mybir
    from concourse.alu_op_type import AluOpType
    from concourse import bass_isa

    C = _host_consts()
    OPS = _custom_ops()
    f32 = mybir.dt.float32
    X = mybir.AxisListType.X
    AF = mybir.ActivationFunctionType

    nc = bacc.Bacc()

    img_e = nc.declare_dram_parameter("img", [3, H, W], f32, isOutput=False)
    p1_e = nc.declare_dram_parameter("p1", [128, 4 * TW * 4 * 2], f32, isOutput=False)
    p0_e = nc.declare_dram_parameter("p0", [128, 4 * TW * 2], f32, isOutput=False)
    maskl_e = nc.declare_dram_parameter("maskl", [128, NT], f32, isOutput=False)
    allbands_e = nc.declare_dram_parameter("allbands", [13, 128, 128], f32, isOutput=False)

    dens_e = nc.declare_dram_parameter("density", [H, W], f32, isOutput=True)
    w0_e = nc.declare_dram_parameter("w0", [128, 4 * TW], f32, isOutput=True)
    w1_e = nc.declare_dram_parameter("w1", [128, 4 * TW], f32, isOutput=True)
    w2_e = nc.declare_dram_parameter("w2", [128, 4 * TW], f32, isOutput=True)
    cout_e = nc.declare_dram_parameter("coords", [128, 4 * TW * 4 * 2], f32, isOutput=True)

    dcols_d = nc.dram_tensor("dcols_scratch", [H, TW], f32)

    with tile.TileContext(nc) as tc:
        with tc.tile_pool(name="persist", bufs=1) as pp:
            # constants (one packed load, issued off the critical sync queue)
            bandt = pp.tile([128, 13, 128], f32, tag="bandt")
            nc.scalar.dma_start(
                bandt[:], allbands_e[:].rearrange("n p j -> p n j"))
            bt = {nm: bandt[:, i, :] for i, nm in enumerate(
                ("band_v2", "band_2v2", "band_121", "band_l"))}
            gbt = [[bandt[:, 4 + gi * 3 + vi, :] for vi in range(3)]
                   for gi in range(3)]
            masklt = pp.tile([128, NT], f32, tag="masklt")
            nc.scalar.dma_start(masklt[:], maskl_e[:])
            cbias = pp.tile([128, 4], f32, tag="cbias")
            nc.vector.memset(cbias[:, 0:1], 2.5e-9)
            nc.vector.memset(cbias[:, 1:2], 4.0e-10)
            nc.vector.memset(cbias[:, 2:3], MIN_D)

            maxstack = pp.tile([128, NT], f32, tag="maxstack")
            minstack = pp.tile([128, NT], f32, tag="minstack")

            shp = tc.tile_pool(name="shp", bufs=1)
            shpool = shp.__enter__()
            CONT = shpool.tile([128, NT, W], f32, tag="CONT")
            scratch = shpool.tile([128, 4096], f32, tag="scratch")

            # ---------------- phase I: gray -> contrast ---------------------
            with tc.tile_pool(name="ph1", bufs=2) as wp, \
                 tc.tile_pool(name="io", bufs=3) as iop, \
                 tc.tile_pool(name="ps1", bufs=1, space="PSUM") as psp:
                for w in range(NT):
                    row0 = STRIDE * w - PAD
                    lo, hi = max(row0, 0), min(row0 + 128, H)
                    plo, phi = lo - row0, hi - row0

                    rgb = iop.tile([128, 3, W], f32, tag="rgb")
                    if plo > 0 or phi < 128:
                        nc.vector.memset(rgb[:], 0.0)
                    nc.sync.dma_start(
                        rgb[plo:phi, :, :],
                        img_e[:, lo:hi, :].rearrange("c r j -> r c j"))
                    R, G, Bc = rgb[:, 0, :], rgb[:, 1, :], rgb[:, 2, :]

                    t0 = wp.tile([128, W], f32, tag="t0")
                    nc.vector.scalar_tensor_tensor(
                        t0[:], R, WR / WG, G,
                        op0=AluOpType.mult, op1=AluOpType.add)
                    g = wp.tile([128, W], f32, tag="g")
                    nc.vector.affine_then_add(g[:], Bc, t0[:], WB / WG, 0.0)

                    b = wp.tile([128, W], f32, tag="b")
                    nc.gpsimd.tensor_tensor(
                        b[:, 1:1023], g[:, 0:1022], g[:, 2:1024], op=AluOpType.add)
                    nc.scalar.copy(b[:, 0:1], g[:, 1:2])
                    nc.scalar.copy(b[:, 1023:1024], g[:, 1022:1023])

                    hd = wp.tile([128, W], f32, tag="hd")
                    nc.gpsimd.tensor_tensor(
                        hd[:, 1:1023], g[:, 2:1024], g[:, 0:1022],
                        op=AluOpType.subtract)
                    nc.scalar.copy(hd[:, 0:1], g[:, 1:2])
                    nc.scalar.mul(hd[:, 1023:1024], g[:, 1022:1023], -1.0)

                    v2_ps = psp.tile([128, W], f32, tag="v2_ps")
                    gy_ps = psp.tile([128, W], f32, tag="gy_ps")
                    gx_ps = psp.tile([128, W], f32, tag="gx_ps")
                    q1_ps = psp.tile([128, W], f32, tag="q1_ps")
                    for hh in (slice(0, 512), slice(512, 1024)):
                        nc.tensor.matmul(v2_ps[:, hh], bt["band_v2"], g[:, hh])
                        nc.tensor.matmul(gy_ps[:, hh], bt["band_2v2"], g[:, hh],
                                         start=True, stop=False)
                        nc.tensor.matmul(gy_ps[:, hh], bt["band_v2"], b[:, hh],
                                         start=False, stop=True)
                        nc.tensor.matmul(gx_ps[:, hh], bt["band_121"], hd[:, hh])
                        nc.tensor.matmul(q1_ps[:, hh], bt["band_l"], g[:, hh])

                    gxs = wp.tile([128, W], f32, tag="gxs")
                    nc.scalar.copy(gxs[:], gx_ps[:])

                    sob = wp.tile([128, W], f32, tag="sob")
                    nc.vector._custom_dve(OPS["SQSUM"], out=sob[:], in0=gxs[:], in1=gy_ps[:])
                    lapl = wp.tile([128, W], f32, tag="lapl")
                    nc.vector._custom_dve(
                        OPS["ABSSUBS"], out=lapl[:], in0=q1_ps[:], in1=b[:],
                        imm2=0.3 * WG)

                    grad = wp.tile([128, W], f32, tag="grad")
                    nc.vector._custom_dve(OPS["SQSUM"], out=grad[:], in0=hd[:], in1=v2_ps[:])
                    nc.vector._custom_dve(
                        OPS["SQSUM"], out=grad[:, 0:1], in0=hd[:, 1:2], in1=v2_ps[:, 0:1])
                    nc.vector._custom_dve(
                        OPS["SQSUM"], out=grad[:, 1023:1024], in0=hd[:, 1022:1023],
                        in1=v2_ps[:, 1023:1024])
                    if w == 0 or w == NT - 1:
                        p = PAD if w == 0 else (1023 - row0)
                        pa, pb = (p + 2, p) if w == 0 else (p, p - 2)
                        nc.gpsimd.dma_start(scratch[0:1, 0:1024], hd[p:p + 1, :])
                        nc.scalar.copy(scratch[0:1, 0:1], scratch[0:1, 1:2])
                        nc.scalar.copy(scratch[0:1, 1023:1024], scratch[0:1, 1022:1023])
                        nc.gpsimd.dma_start(scratch[0:1, 1024:2048], g[pa:pa + 1, :])
                        nc.gpsimd.dma_start(scratch[0:1, 3072:4096], g[pb:pb + 1, :])
                        nc.vector.tensor_tensor(
                            scratch[0:1, 1024:2048], scratch[0:1, 1024:2048],
                            scratch[0:1, 3072:4096], op=AluOpType.subtract)
                        nc.vector._custom_dve(
                            OPS["SQSUM"], out=scratch[0:1, 2048:3072],
                            in0=scratch[0:1, 0:1024], in1=scratch[0:1, 1024:2048])
                        nc.gpsimd.dma_start(grad[p:p + 1, :], scratch[0:1, 2048:3072])

                    sob_s = wp.tile([128, W], f32, tag="sob_s")
                    nc.scalar.activation(
                        sob_s[:], sob[:], AF.Sqrt,
                        bias=cbias[:, 0:1], scale=0.25 * WG * WG)
                    grad_s = wp.tile([128, W], f32, tag="grad_s")
                    nc.scalar.activation(
                        grad_s[:], grad[:], AF.Sqrt,
                        bias=cbias[:, 1:2], scale=0.04 * WG * WG)

                    t2 = wp.tile([128, W], f32, tag="t2")
                    nc.gpsimd.tensor_tensor(t2[:], sob_s[:], lapl[:], op=AluOpType.add)

                    nc.vector._custom_dve(
                        OPS["ADD_ACCMAX"], out=CONT[:, w, :], in0=t2[:], in1=grad_s[:],
                        s0=-LARGE, accum_out=maxstack[:, w:w + 1])
                    nc.vector.tensor_reduce(
                        minstack[:, w:w + 1], CONT[:, w, :], axis=X, op=AluOpType.min)

            # ---------------- min/max aggregation ---------------------------
            mm1 = pp.tile([128, NT], f32, tag="mm1")
            nc.vector.tensor_tensor(mm1[:], maxstack[:], masklt[:], op=AluOpType.subtract)
            mm2 = pp.tile([128, NT], f32, tag="mm2")
            nc.vector.tensor_tensor(mm2[:], minstack[:], masklt[:], op=AluOpType.add)
            mxr = pp.tile([128, 1], f32, tag="mxr")
            nc.vector.tensor_reduce(mxr[:], mm1[:], axis=X, op=AluOpType.max)
            mnr = pp.tile([128, 1], f32, tag="mnr")
            nc.vector.tensor_reduce(mnr[:], mm2[:], axis=X, op=AluOpType.min)
            negmn = pp.tile([128, 1], f32, tag="negmn")
            nc.vector.tensor_single_scalar(negmn[:], mnr[:], -1.0, op=AluOpType.mult)
            mxa = pp.tile([128, 1], f32, tag="mxa")
            nc.gpsimd.partition_all_reduce(mxa[:], mxr[:], channels=128,
                                           reduce_op=bass_isa.ReduceOp.max)
            nga = pp.tile([128, 1], f32, tag="nga")
            nc.gpsimd.partition_all_reduce(nga[:], negmn[:], channels=128,
                                           reduce_op=bass_isa.ReduceOp.max)
            rng = pp.tile([128, 1], f32, tag="rng")
            nc.vector.tensor_tensor(rng[:], mxa[:], nga[:], op=AluOpType.add)
            inv = pp.tile([128, 1], f32, tag="inv")
            nc.vector.reciprocal(inv[:], rng[:])
            mnv = pp.tile([128, 1], f32, tag="mnv")
            nc.vector.tensor_single_scalar(mnv[:], nga[:], -1.0, op=AluOpType.mult)

            # ---------------- phase II: normalize + gauss + density ---------
            with tc.tile_pool(name="ph2", bufs=2) as wp2, \
                 tc.tile_pool(name="ps2", bufs=2, space="PSUM") as ps2:
                for w in range(NT):
                    vi = 0 if w == 0 else (2 if w == NT - 1 else 1)
                    nrm = wp2.tile([128, W], f32, tag="nrm")
                    nc.vector.tensor_scalar(
                        nrm[:], CONT[:, w, :], mnv[:], inv[:],
                        op0=AluOpType.subtract, op1=AluOpType.mult)
                    h1 = wp2.tile([128, W], f32, tag="h1")
                    nc.gpsimd.tensor_tensor(
                        h1[:, 1:1023], nrm[:, 0:1022], nrm[:, 2:1024], op=AluOpType.add)
                    nc.scalar.copy(h1[:, 0:1], nrm[:, 1:2])
                    nc.scalar.copy(h1[:, 1023:1024], nrm[:, 1022:1023])
                    h2 = wp2.tile([128, W], f32, tag="h2")
                    nc.gpsimd.tensor_tensor(
                        h2[:, 2:1022], nrm[:, 0:1020], nrm[:, 4:1024], op=AluOpType.add)
                    nc.scalar.copy(h2[:, 0:2], nrm[:, 2:4])
                    nc.scalar.copy(h2[:, 1022:1024], nrm[:, 1020:1022])

                    x_ps = ps2.tile([128, W], f32, tag="x_ps")
                    for hh in (slice(0, 512), slice(512, 1024)):
                        nc.tensor.matmul(x_ps[:, hh], gbt[0][vi], nrm[:, hh],
                                         start=True, stop=False)
                        nc.tensor.matmul(x_ps[:, hh], gbt[1][vi], h2[:, hh],
                                         start=False, stop=False)
                        nc.tensor.matmul(x_ps[:, hh], gbt[2][vi], h1[:, hh],
                                         start=False, stop=True)

                    r = wp2.tile([128, W], f32, tag="r")
                    nc.scalar.activation(r[:], x_ps[:], AF.Relu, scale=SPAN * SPAN)
                    qq = wp2.tile([128, W], f32, tag="qq")
                    nc.scalar.activation(qq[:], r[:], AF.Sqrt)
                    dens = wp2.tile([128, W], f32, tag="dens")
                    nc.scalar.activation(dens[:], qq[:], AF.Identity, bias=cbias[:, 2:3])
                    dcl = wp2.tile([128, TW], f32, tag="dcl")
                    nc.scalar.copy(dcl[:], dens[:, 0:1024:2])
                    lo_v = STRIDE * w
                    hi_v = min(lo_v + STRIDE, H)
                    pv0, pv1 = PAD, PAD + (hi_v - lo_v)
                    eng = (nc.sync, nc.scalar, nc.gpsimd)[w % 3]
                    eng.dma_start(dens_e[lo_v:hi_v, :], dens[pv0:pv1, :])
                    eng2 = (nc.scalar, nc.gpsimd, nc.sync)[w % 3]
                    eng2.dma_start(dcols_d[lo_v:hi_v, :], dcl[pv0:pv1, :])

            shp.__exit__(None, None, None)

            # ---------------- sample stage: weights + coords ----------------
            with tc.tile_pool(name="samp", bufs=1) as sp, \
                 tc.tile_pool(name="cq", bufs=2) as cqp:
                cqt = sp.tile([128, 4 * TW * 4 * 2], f32, tag="cqt")
                for qi, qe in enumerate((nc.sync, nc.scalar, nc.gpsimd, nc.scalar)):
                    qe.dma_start(cqt[:, qi * 4096:(qi + 1) * 4096],
                                 p1_e[:, qi * 4096:(qi + 1) * 4096])
                D = sp.tile([128, 4, TW], f32, tag="D")
                nc.sync.dma_start(
                    D[:], dcols_d[:].rearrange("(q s) j -> q s j", s=8)[:, 0:8:2, :])
                Dv = D[:].rearrange("p s j -> p (s j)")
                m2 = sp.tile([128, 4 * TW], f32, tag="m2")
                nc.vector.tensor_single_scalar(m2[:], Dv, 0.4, op=AluOpType.is_gt)
                m2i = sp.tile([128, 4 * TW], mybir.dt.uint8, tag="m2i")
                nc.vector.tensor_single_scalar(m2i[:], Dv, 0.4, op=AluOpType.is_le)
                m4 = sp.tile([128, 4 * TW], f32, tag="m4")
                nc.vector.tensor_single_scalar(m4[:], Dv, 0.7, op=AluOpType.is_gt)
                wq = sp.tile([128, 4 * TW], f32, tag="wq")
                nc.vector._custom_dve(
                    OPS["WQ"], out=wq[:], in0=Dv, in1=Dv, s0=0.4, s1=0.7, imm2=0.5)
                w1t = sp.tile([128, 4 * TW], f32, tag="w1t")
                nc.vector.tensor_tensor(w1t[:], wq[:], m2[:], op=AluOpType.mult)
                w2t = sp.tile([128, 4 * TW], f32, tag="w2t")
                nc.vector.tensor_tensor(w2t[:], wq[:], m4[:], op=AluOpType.mult)
                nc.sync.dma_start(w0_e[:], wq[:])
                nc.scalar.dma_start(w1_e[:], w1t[:])
                nc.gpsimd.dma_start(w2_e[:], w2t[:])

                p0t = sp.tile([128, 4 * TW * 2], f32, tag="p0t")
                nc.sync.dma_start(p0t[:], p0_e[:])
                # cqt is prefilled with p1 = base+off; overwrite base where ns==1
                cqv = cqt[:].rearrange("p (n s c) -> p n s c", s=4, c=2)
                p0v = p0t[:].rearrange("p (n c) -> p n c", c=2)
                for s_ in range(4):
                    for c_ in range(2):
                        nc.vector.copy_predicated(
                            cqv[:, :, s_, c_], m2i[:], p0v[:, :, c_])
                for r_ in range(4):
                    eng = (nc.sync, nc.scalar, nc.gpsimd, nc.sync)[r_]
                    eng.dma_start(
                        cout_e[:, r_ * 4096:(r_ + 1) * 4096],
                        cqt[:, r_ * 4096:(r_ + 1) * 4096])

    nc.finalize()
    return nc


def kernel(img, target_height, target_width, **_kw):
    global LAST_EXEC_TIME_NS, LAST_RESULT
    th, tw = int(target_height), int(target_width)
    img = np.ascontiguousarray(np.asarray(img, dtype=np.float32))
    assert img.shape == (B, 3, H, W) and th == TH and tw == TW, (
        f"kernel hardcoded for img(8,3,1024,1024), th=tw=512; got {img.shape} {th} {tw}")

    if TRACE:
        _ensure_axon_hooks()

    C = _host_consts()
    nc = _build()

    from concourse.bass_utils import run_bass_kernel_spmd

    in_maps = []
    for i in range(B):
        m = {
            "img": img[i],
            "p1": C["p1_all"][i],
            "p0": C["p0"],
            "maskl": C["maskl"],
            "allbands": C["allbands"],
        }
        in_maps.append(m)
    res = run_bass_kernel_spmd(
        nc, in_maps, core_ids=list(range(B)), trace=TRACE)
    LAST_EXEC_TIME_NS = res.exec_time_ns
    LAST_RESULT = res

    coords = np.empty((B, TH * TW, 4, 2), np.float32)
    weights = np.empty((B, TH * TW, 4), np.float32)
    density = np.empty((B, 1, H, W), np.float32)
    for i in range(B):
        r = res.results[i]
        coords[i] = r["coords"].reshape(TH * TW, 4, 2)
        w0 = r["w0"].reshape(-1)
        w1 = r["w1"].reshape(-1)
        w2 = r["w2"].reshape(-1)
        weights[i] = np.stack([w0, w1, w2, w2], axis=-1)
        density[i, 0] = r["density"]
    return coords, weights, density


# revision 21
# speedup vs baseline: 1.0482x; 1.0395x over previous
"""AdaptiveINR Trainium2 kernel (8-core data parallel).

Pipeline per image (one image per NeuronCore):
  contrast  = 0.5*sobel + 0.3*|lapl| + 0.2*gradm    (3x3 stencils on gray)
  density   = 0.1 + 0.9*sqrt(gauss5x5(minmax_norm(contrast)))
  coords    = base + (d>0.4)*off ;  weights = where(s<ns, d/ns, 0)
where d = density sampled at even rows/cols, off = input-independent
jax.random offsets (key 42) precomputed on host CPU.

Layout: 9 overlapping row-tiles per image, tile w holds image rows
122*w-3 .. 122*w+124 in partitions 0..127 (out-of-range rows zeroed).
Vertical convs (and the vertical half of the separable gauss, with the
horizontal taps accumulated in PSUM) are banded matmuls on TensorE;
horizontal shifts are shifted-AP ops on VectorE/GpSimd; transcendentals
on ScalarE.
"""

import functools
import os
import sys
import types

import numpy as np

B, H, W = 8, 1024, 1024
TH, TW = 512, 512
NT = 9          # row tiles per image
STRIDE = 122    # valid rows per tile
PAD = 3         # halo rows above the valid range

WR, WG, WB = 0.299, 0.587, 0.114
MIN_D, MAX_D = 0.1, 1.0
SPAN = MAX_D - MIN_D  # 0.9
LARGE = 1e30

TRACE = bool(os.environ.get("BASS_KERNEL_TRACE"))
LAST_EXEC_TIME_NS = None
LAST_RESULT = None


def _ensure_axon_hooks():
    """Register the NTFF profile hook (missing antenv.axon_hooks on this image)."""
    if "antenv.axon_hooks" in sys.modules:
        return
    try:
        import antenv  # noqa: F401

        mod = types.ModuleType("antenv.axon_hooks")
        _hook = {}
        mod.set_axon_ntff_profile_hook = lambda h: _hook.__setitem__("h", h)
        mod.get_axon_ntff_profile_hook = lambda: _hook.get("h")
        sys.modules["antenv.axon_hooks"] = mod
        from trn_agent_boot.trn_boot import _ntff_profile_via_ctypes

        mod.set_axon_ntff_profile_hook(
            _ntff_profile_via_ctypes("/opt/axon/libaxon_pjrt.so")
        )
    except Exception:
        pass


def _gauss1d():
    sigma = 5 / 6.0
    r = np.arange(5, dtype=np.float32) - 2
    k1 = np.exp(-0.5 * r**2 / sigma**2)
    k1 = (k1 / k1.sum()).astype(np.float32)
    return k1


@functools.lru_cache(maxsize=1)
def _host_consts():
    import jax
    import jax.numpy as jnp

    with jax.default_device(jax.devices("cpu")[0]):
        u = jax.random.uniform(jax.random.key(42), (B, TH, TW, 4, 2), jnp.float32)
        off = np.asarray((u - 0.5) * 0.8 * (2.0 / TH)).astype(np.float32)

    y = np.linspace(-1.0, 1.0, TH, dtype=np.float32)
    x = np.linspace(-1.0, 1.0, TW, dtype=np.float32)
    base = np.empty((TH, TW, 2), np.float32)
    base[:, :, 0] = y[:, None]
    base[:, :, 1] = x[None, :]
    p1b = (base[None, :, :, None, :] + off).astype(np.float32)  # [B, TH, TW, 4, 2]
    p1_all = np.ascontiguousarray(
        p1b.reshape(B, 128, 4, TW, 4, 2).reshape(B, 128, 4 * TW * 4 * 2))

    p0 = np.empty((128, 4, TW, 2), np.float32)
    p0[:, :, :, 0] = y.reshape(128, 4, 1)
    p0[:, :, :, 1] = x.reshape(1, 1, TW)
    p0 = np.ascontiguousarray(p0.reshape(128, 4 * TW * 2))

    # band matrices: band[k, m] = coeff of input partition k for output m
    def band(coeffs, zero_lo=0, zero_hi=0):
        bm = np.zeros((128, 128), np.float32)
        for dk, c in coeffs.items():
            for m in range(128):
                k = m + dk
                if 0 <= k < 128:
                    bm[k, m] = c
        if zero_lo:
            bm[:zero_lo, :] = 0.0
        if zero_hi:
            bm[-zero_hi:, :] = 0.0
        return bm

    k1 = _gauss1d()
    g0, g1, g2 = float(k1[0]), float(k1[1]), float(k1[2])
    band_v2 = band({-1: -1.0, 1: 1.0})
    band_2v2 = band({-1: -2.0, 1: 2.0})
    band_121 = band({-1: 1.0, 0: 2.0, 1: 1.0})
    band_l = band({-1: -1.0, 0: 4.0, 1: -1.0})
    gbands = []
    gb1 = {dk: float(k1[dk + 2]) * g1 for dk in range(-2, 3)}
    for scale in (g2 / g1, g0 / g1, 1.0):  # applied to nrm, h2, h1
        gb = {dk: v * scale for dk, v in gb1.items()}
        gbands.append([
            band(gb, zero_lo=PAD),           # tile 0
            band(gb),                        # middle
            band(gb, zero_hi=128 - 51),      # tile 8 (partitions >=51 are rows >=1024)
        ])

    # untrusted (non-duplicate, non-valid) rows for the min/max reduction
    maskl = np.zeros((128, NT), np.float32)
    maskl[0, :] = LARGE
    maskl[127, :] = LARGE
    maskl[1:3, 0] = LARGE
    maskl[51:, NT - 1] = LARGE

    allbands = np.ascontiguousarray(np.stack(
        [band_v2, band_2v2, band_121, band_l]
        + [gbands[gi][vi] for gi in range(3) for vi in range(3)], axis=0))
    return dict(
        p1_all=p1_all, p0=p0, maskl=np.ascontiguousarray(maskl),
        allbands=allbands, g0=g0, g1=g1, g2=g2,
    )


@functools.lru_cache(maxsize=1)
def _custom_ops():
    import concourse.dve_ops as dve_ops
    from concourse.dve_spec import Spec, Src0, Src1, C0, C1, C2, Zero, lower, sq, maxx
    from concourse.dve_uop import DveOpSpec

    def author(name, spec, rd1=True):
        if name in dve_ops._SUB_OPCODE_FOR_NAME:
            for op in dve_ops.OPS:
                if op.name == name:
                    return op
        row = max(dve_ops._SUB_OPCODE_FOR_NAME.values()) + 1
        dve_ops._SUB_OPCODE_FOR_NAME[name] = row
        shas = {}
        for ver in ("v3", "v4"):
            uops = lower(spec, ver=ver)
            shas[ver] = DveOpSpec(name=name, opcode=row, uops=uops, rd1_en=rd1).sha(ver)
        op = dve_ops.DveOp(name, spec, subdim=False, uops_sha=shas)
        dve_ops.OPS.append(op)
        dve_ops.CUSTOM_DVE_SPECS[name] = spec
        return op

    SQSUM = author("ANT_SQSUM", Spec(
        body=sq(Src0) + sq(Src1),
        reference=lambda i0, i1, s0, s1, c2: (
            i0.astype(np.float32) ** 2 + i1.astype(np.float32) ** 2
        ),
    ))
    body_abs = maxx(Src0 - Src1, Zero - (Src0 - Src1)) * C2
    ABSSUBS = author("ANT_ABSSUBS", Spec(
        body=body_abs,
        reference=lambda i0, i1, s0, s1, c2: (np.abs(
            i0.astype(np.float32) - i1.astype(np.float32)) * c2),
    ))
    ADD_ACCMAX = author("ANT_ADD_ACCMAX", Spec(
        body=Src0 + Src1, accum=maxx, accum_init=C0,
        reference=lambda i0, i1, s0, s1, c2: (
            (i0 + i1).astype(np.float32),
            np.maximum(s0, (i0 + i1).max(axis=-1, keepdims=True)).astype(np.float32),
        ),
    ))
    body_wq = Src0 - C2 * ((Src1 > C0) * Src0 + C2 * ((Src0 > C1) * Src0))
    WQ = author("ANT_WQ", Spec(
        body=body_wq,
        reference=lambda i0, i1, s0, s1, c2: (
            i0 - (i0 * c2) * (i0 > s0) - ((i1 * c2) * c2) * (i1 > s1)
        ).astype(np.float32),
    ))
    return dict(SQSUM=SQSUM, ABSSUBS=ABSSUBS, ADD_ACCMAX=ADD_ACCMAX, WQ=WQ)


@functools.lru_cache(maxsize=1)
def _build():
    import concourse.bass as bass
    import concourse.tile as tile
    from concourse import bacc, mybir
    from concourse.alu_op_type import AluOpType
    from concourse import bass_isa

    C = _host_consts()
    OPS = _custom_ops()
    f32 = mybir.dt.float32
    X = mybir.AxisListType.X
    AF = mybir.ActivationFunctionType

    nc = bacc.Bacc()

    img_e = nc.declare_dram_parameter("img", [3, H, W], f32, isOutput=False)
    p1_e = nc.declare_dram_parameter("p1", [128, 4 * TW * 4 * 2], f32, isOutput=False)
    p0_e = nc.declare_dram_parameter("p0", [128, 4 * TW * 2], f32, isOutput=False)
    maskl_e = nc.declare_dram_parameter("maskl", [128, NT], f32, isOutput=False)
    allbands_e = nc.declare_dram_parameter("allbands", [13, 128, 128], f32, isOutput=False)

    dens_e = nc.declare_dram_parameter("density", [H, W], f32, isOutput=True)
    w0_e = nc.declare_dram_parameter("w0", [128, 4 * TW], f32, isOutput=True)
    w1_e = nc.declare_dram_parameter("w1", [128, 4 * TW], f32, isOutput=True)
    w2_e = nc.declare_dram_parameter("w2", [128, 4 * TW], f32, isOutput=True)
    cout_e = nc.declare_dram_parameter("coords", [128, 4 * TW * 4 * 2], f32, isOutput=True)

    dcols_d = nc.dram_tensor("dcols_scratch", [H, TW], f32)

    with tile.TileContext(nc) as tc:
        with tc.tile_pool(name="persist", bufs=1) as pp:
            # constants (one packed load, issued off the critical sync queue)
            bandt = pp.tile([128, 13, 128], f32, tag="bandt")
            nc.scalar.dma_start(
                bandt[:], allbands_e[:].rearrange("n p j -> p n j"))
            bt = {nm: bandt[:, i, :] for i, nm in enumerate(
                ("band_v2", "band_2v2", "band_121", "band_l"))}
            gbt = [[bandt[:, 4 + gi * 3 + vi, :] for vi in range(3)]
                   for gi in range(3)]
            masklt = pp.tile([128, NT], f32, tag="masklt")
            nc.scalar.dma_start(masklt[:], maskl_e[:])
            cbias = pp.tile([128, 4], f32, tag="cbias")
            nc.vector.memset(cbias[:, 0:1], 2.5e-9)
            nc.vector.memset(cbias[:, 1:2], 4.0e-10)
            nc.vector.memset(cbias[:, 2:3], MIN_D)

            maxstack = pp.tile([128, NT], f32, tag="maxstack")
            minstack = pp.tile([128, NT], f32, tag="minstack")

            shp = tc.tile_pool(name="shp", bufs=1)
            shpool = shp.__enter__()
            CONT = shpool.tile([128, NT, W], f32, tag="CONT")
            scratch = shpool.tile([128, 4096], f32, tag="scratch")

            # ---------------- phase I: gray -> contrast ---------------------
            with tc.tile_pool(name="ph1", bufs=2) as wp, \
                 tc.tile_pool(name="io", bufs=3) as iop, \
                 tc.tile_pool(name="ps1", bufs=1, space="PSUM") as psp:
                for w in range(NT):
                    row0 = STRIDE * w - PAD
                    lo, hi = max(row0, 0), min(row0 + 128, H)
                    plo, phi = lo - row0, hi - row0

                    rgb = iop.tile([128, 3, W], f32, tag="rgb")
                    if plo > 0 or phi < 128:
                        nc.vector.memset(rgb[:], 0.0)
                    nc.sync.dma_start(
                        rgb[plo:phi, :, :],
                        img_e[:, lo:hi, :].rearrange("c r j -> r c j"))
                    R, G, Bc = rgb[:, 0, :], rgb[:, 1, :], rgb[:, 2, :]

                    t0 = wp.tile([128, W], f32, tag="t0")
                    nc.vector.scalar_tensor_tensor(
                        t0[:], R, WR / WG, G,
                        op0=AluOpType.mult, op1=AluOpType.add)
                    g = wp.tile([128, W], f32, tag="g")
                    nc.vector.affine_then_add(g[:], Bc, t0[:], WB / WG, 0.0)

                    b = wp.tile([128, W], f32, tag="b")
                    nc.gpsimd.tensor_tensor(
                        b[:, 1:1023], g[:, 0:1022], g[:, 2:1024], op=AluOpType.add)
                    nc.scalar.copy(b[:, 0:1], g[:, 1:2])
                    nc.scalar.copy(b[:, 1023:1024], g[:, 1022:1023])

                    hd = wp.tile([128, W], f32, tag="hd")
                    nc.gpsimd.tensor_tensor(
                        hd[:, 1:1023], g[:, 2:1024], g[:, 0:1022],
                        op=AluOpType.subtract)
                    nc.scalar.copy(hd[:, 0:1], g[:, 1:2])
                    nc.scalar.mul(hd[:, 1023:1024], g[:, 1022:1023], -1.0)

                    v2_ps = psp.tile([128, W], f32, tag="v2_ps")
                    gy_ps = psp.tile([128, W], f32, tag="gy_ps")
                    gx_ps = psp.tile([128, W], f32, tag="gx_ps")
                    q1_ps = psp.tile([128, W], f32, tag="q1_ps")
                    for hh in (slice(0, 512), slice(512, 1024)):
                        nc.tensor.matmul(v2_ps[:, hh], bt["band_v2"], g[:, hh])
                        nc.tensor.matmul(gy_ps[:, hh], bt["band_2v2"], g[:, hh],
                                         start=True, stop=False)
                        nc.tensor.matmul(gy_ps[:, hh], bt["band_v2"], b[:, hh],
                                         start=False, stop=True)
                        nc.tensor.matmul(gx_ps[:, hh], bt["band_121"], hd[:, hh])
                        nc.tensor.matmul(q1_ps[:, hh], bt["band_l"], g[:, hh])

                    gxs = wp.tile([128, W], f32, tag="gxs")
                    nc.scalar.copy(gxs[:], gx_ps[:])

                    sob = wp.tile([128, W], f32, tag="sob")
                    nc.vector._custom_dve(OPS["SQSUM"], out=sob[:], in0=gxs[:], in1=gy_ps[:])
                    lapl = wp.tile([128, W], f32, tag="lapl")
                    nc.vector._custom_dve(
                        OPS["ABSSUBS"], out=lapl[:], in0=q1_ps[:], in1=b[:],
                        imm2=0.3 * WG)

                    grad = wp.tile([128, W], f32, tag="grad")
                    nc.vector._custom_dve(OPS["SQSUM"], out=grad[:], in0=hd[:], in1=v2_ps[:])
                    nc.vector._custom_dve(
                        OPS["SQSUM"], out=grad[:, 0:1], in0=hd[:, 1:2], in1=v2_ps[:, 0:1])
                    nc.vector._custom_dve(
                        OPS["SQSUM"], out=grad[:, 1023:1024], in0=hd[:, 1022:1023],
                        in1=v2_ps[:, 1023:1024])
                    if w == 0 or w == NT - 1:
                        p = PAD if w == 0 else (1023 - row0)
                        pa, pb = (p + 2, p) if w == 0 else (p, p - 2)
                        nc.gpsimd.dma_start(scratch[0:1, 0:1024], hd[p:p + 1, :])
                        nc.scalar.copy(scratch[0:1, 0:1], scratch[0:1, 1:2])
                        nc.scalar.copy(scratch[0:1, 1023:1024], scratch[0:1, 1022:1023])
                        nc.gpsimd.dma_start(scratch[0:1, 1024:2048], g[pa:pa + 1, :])
                        nc.gpsimd.dma_start(scratch[0:1, 3072:4096], g[pb:pb + 1, :])
                        nc.vector.tensor_tensor(
                            scratch[0:1, 1024:2048], scratch[0:1, 1024:2048],
                            scratch[0:1, 3072:4096], op=AluOpType.subtract)
                        nc.vector._custom_dve(
                            OPS["SQSUM"], out=scratch[0:1, 2048:3072],
                            in0=scratch[0:1, 0:1024], in1=scratch[0:1, 1024:2048])
                        nc.gpsimd.dma_start(grad[p:p + 1, :], scratch[0:1, 2048:3072])

                    sob_s = wp.tile([128, W], f32, tag="sob_s")
                    nc.scalar.activation(
                        sob_s[:], sob[:], AF.Sqrt,
                        bias=cbias[:, 0:1], scale=0.25 * WG * WG)
                    grad_s = wp.tile([128, W], f32, tag="grad_s")
                    nc.scalar.activation(
                        grad_s[:], grad[:], AF.Sqrt,
                        bias=cbias[:, 1:2], scale=0.04 * WG * WG)

                    t2 = wp.tile([128, W], f32, tag="t2")
                    nc.gpsimd.tensor_tensor(t2[:], sob_s[:], lapl[:], op=AluOpType.add)

                    nc.vector._custom_dve(
                        OPS["ADD_ACCMAX"], out=CONT[:, w, :], in0=t2[:], in1=grad_s[:],
                        s0=-LARGE, accum_out=maxstack[:, w:w + 1])
                    nc.vector.tensor_reduce(
                        minstack[:, w:w + 1], CONT[:, w, :], axis=X, op=AluOpType.min)

            # ---------------- min/max aggregation ---------------------------
            mm1 = pp.tile([128, NT], f32, tag="mm1")
            nc.vector.tensor_tensor(mm1[:], maxstack[:], masklt[:], op=AluOpType.subtract)
            mm2 = pp.tile([128, NT], f32, tag="mm2")
            nc.vector.tensor_tensor(mm2[:], minstack[:], masklt[:], op=AluOpType.add)
            mxr = pp.tile([128, 1], f32, tag="mxr")
            nc.vector.tensor_reduce(mxr[:], mm1[:], axis=X, op=AluOpType.max)
            mnr = pp.tile([128, 1], f32, tag="mnr")
            nc.vector.tensor_reduce(mnr[:], mm2[:], axis=X, op=AluOpType.min)
            negmn = pp.tile([128, 1], f32, tag="negmn")
            nc.vector.tensor_single_scalar(negmn[:], mnr[:], -1.0, op=AluOpType.mult)
            mxa = pp.tile([128, 1], f32, tag="mxa")
            nc.gpsimd.partition_all_reduce(mxa[:], mxr[:], channels=128,
                                           reduce_op=bass_isa.ReduceOp.max)
            nga = pp.tile([128, 1], f32, tag="nga")
            nc.gpsimd.partition_all_reduce(nga[:], negmn[:], channels=128,
                                           reduce_op=bass_isa.ReduceOp.max)
            rng = pp.tile([128, 1], f32, tag="rng")
            nc.vector.tensor_tensor(rng[:], mxa[:], nga[:], op=AluOpType.add)
            inv = pp.tile([128, 1], f32, tag="inv")
            nc.vector.reciprocal(inv[:], rng[:])
            mnv = pp.tile([128, 1], f32, tag="mnv")
            nc.vector.tensor_single_scalar(mnv[:], nga[:], -1.0, op=AluOpType.mult)

            # ---------------- phase II: normalize + gauss + density ---------
            with tc.tile_pool(name="ph2", bufs=2) as wp2, \
                 tc.tile_pool(name="ps2", bufs=2, space="PSUM") as ps2:
                for w in range(NT):
                    vi = 0 if w == 0 else (2 if w == NT - 1 else 1)
                    nrm = wp2.tile([128, W], f32, tag="nrm")
                    nc.vector.tensor_scalar(
                        nrm[:], CONT[:, w, :], mnv[:], inv[:],
                        op0=AluOpType.subtract, op1=AluOpType.mult)
                    h1 = wp2.tile([128, W], f32, tag="h1")
                    nc.gpsimd.tensor_tensor(
                        h1[:, 1:1023], nrm[:, 0:1022], nrm[:, 2:1024], op=AluOpType.add)
                    nc.scalar.copy(h1[:, 0:1], nrm[:, 1:2])
                    nc.scalar.copy(h1[:, 1023:1024], nrm[:, 1022:1023])
                    h2 = wp2.tile([128, W], f32, tag="h2")
                    nc.gpsimd.tensor_tensor(
                        h2[:, 2:1022], nrm[:, 0:1020], nrm[:, 4:1024], op=AluOpType.add)
                    nc.scalar.copy(h2[:, 0:2], nrm[:, 2:4])
                    nc.scalar.copy(h2[:, 1022:1024], nrm[:, 1020:1022])

                    x_ps = ps2.tile([128, W], f32, tag="x_ps")
                    for hh in (slice(0, 512), slice(512, 1024)):
                        nc.tensor.matmul(x_ps[:, hh], gbt[0][vi], nrm[:, hh],
                                         start=True, stop=False)
                        nc.tensor.matmul(x_ps[:, hh], gbt[1][vi], h2[:, hh],
                                         start=False, stop=False)
                        nc.tensor.matmul(x_ps[:, hh], gbt[2][vi], h1[:, hh],
                                         start=False, stop=True)

                    r = wp2.tile([128, W], f32, tag="r")
                    nc.scalar.activation(r[:], x_ps[:], AF.Relu, scale=SPAN * SPAN)
                    qq = wp2.tile([128, W], f32, tag="qq")
                    nc.scalar.activation(qq[:], r[:], AF.Sqrt)
                    dens = wp2.tile([128, W], f32, tag="dens")
                    nc.scalar.activation(dens[:], qq[:], AF.Identity, bias=cbias[:, 2:3])
                    dcl = wp2.tile([128, TW], f32, tag="dcl")
                    nc.scalar.copy(dcl[:], dens[:, 0:1024:2])
                    lo_v = STRIDE * w
                    hi_v = min(lo_v + STRIDE, H)
                    pv0, pv1 = PAD, PAD + (hi_v - lo_v)
                    eng = (nc.scalar, nc.gpsimd)[w % 2]
                    eng.dma_start(dens_e[lo_v:hi_v, :], dens[pv0:pv1, :])
                    eng2 = (nc.gpsimd, nc.scalar)[w % 2]
                    eng2.dma_start(dcols_d[lo_v:hi_v, :], dcl[pv0:pv1, :])

            shp.__exit__(None, None, None)

            # ---------------- sample stage: weights + coords ----------------
            with tc.tile_pool(name="samp", bufs=1) as sp, \
                 tc.tile_pool(name="cq", bufs=1) as cqp:
                cqt = sp.tile([128, 4 * TW * 4 * 2], f32, tag="cqt")
                for qi in range(4):
                    nc.sync.dma_start(cqt[:, qi * 4096:(qi + 1) * 4096],
                                      p1_e[:, qi * 4096:(qi + 1) * 4096])
                D = sp.tile([128, 4, TW], f32, tag="D")
                nc.sync.dma_start(
                    D[:], dcols_d[:].rearrange("(q s) j -> q s j", s=8)[:, 0:8:2, :])
                Dv = D[:].rearrange("p s j -> p (s j)")
                m2 = sp.tile([128, 4 * TW], f32, tag="m2")
                nc.vector.tensor_single_scalar(m2[:], Dv, 0.4, op=AluOpType.is_gt)
                m2i = sp.tile([128, 4 * TW], mybir.dt.uint8, tag="m2i")
                nc.vector.tensor_single_scalar(m2i[:], Dv, 0.4, op=AluOpType.is_le)
                m4 = sp.tile([128, 4 * TW], f32, tag="m4")
                nc.vector.tensor_single_scalar(m4[:], Dv, 0.7, op=AluOpType.is_gt)
                wq = sp.tile([128, 4 * TW], f32, tag="wq")
                nc.vector._custom_dve(
                    OPS["WQ"], out=wq[:], in0=Dv, in1=Dv, s0=0.4, s1=0.7, imm2=0.5)
                w1t = sp.tile([128, 4 * TW], f32, tag="w1t")
                nc.vector.tensor_tensor(w1t[:], wq[:], m2[:], op=AluOpType.mult)
                w2t = sp.tile([128, 4 * TW], f32, tag="w2t")
                nc.vector.tensor_tensor(w2t[:], wq[:], m4[:], op=AluOpType.mult)
                nc.scalar.dma_start(w0_e[:], wq[:])
                nc.scalar.dma_start(w1_e[:], w1t[:])
                nc.gpsimd.dma_start(w2_e[:], w2t[:])

                p0t = sp.tile([128, 4 * TW * 2], f32, tag="p0t")
                nc.sync.dma_start(p0t[:], p0_e[:])
                m2e8 = sp.tile([128, 4 * TW, 8], mybir.dt.uint8, tag="m2e8")
                nc.gpsimd.tensor_copy(
                    m2e8[:], m2i[:].rearrange(
                        "p (n o) -> p n o", o=1).broadcast_to((128, 4 * TW, 8)))
                # cqt is prefilled with p1 = base+off; overwrite base where ns==1
                for hf in range(2):
                    p0eh = cqp.tile([128, 2 * TW * 4 * 2], f32, tag="p0eh")
                    nc.scalar.copy(
                        p0eh[:].rearrange("p (n s c) -> p n s c", s=4, c=2),
                        p0t[:, hf * 2048:(hf + 1) * 2048].rearrange(
                            "p (n o c) -> p n o c", o=1, c=2)
                        .broadcast_to((128, 2 * TW, 4, 2)))
                    nc.vector.copy_predicated(
                        cqt[:, hf * 8192:(hf + 1) * 8192],
                        m2e8[:, hf * 2 * TW:(hf + 1) * 2 * TW, :].rearrange(
                            "p n o -> p (n o)"),
                        p0eh[:])
                for r_ in range(4):
                    eng = (nc.sync, nc.scalar, nc.gpsimd, nc.sync)[r_]
                    eng.dma_start(
                        cout_e[:, r_ * 4096:(r_ + 1) * 4096],
                        cqt[:, r_ * 4096:(r_ + 1) * 4096])

    nc.finalize()
    return nc


def kernel(img, target_height, target_width, **_kw):
    global LAST_EXEC_TIME_NS, LAST_RESULT
    th, tw = int(target_height), int(target_width)
    img = np.ascontiguousarray(np.asarray(img, dtype=np.float32))
    assert img.shape == (B, 3, H, W) and th == TH and tw == TW, (
        f"kernel hardcoded for img(8,3,1024,1024), th=tw=512; got {img.shape} {th} {tw}")

    if TRACE:
        _ensure_axon_hooks()

    C = _host_consts()
    nc = _build()

    from concourse.bass_utils import run_bass_kernel_spmd

    in_maps = []
    for i in range(B):
        m = {
            "img": img[i],
            "p1": C["p1_all"][i],
            "p0": C["p0"],
            "maskl": C["maskl"],
            "allbands": C["allbands"],
        }
        in_maps.append(m)
    res = run_bass_kernel_spmd(
        nc, in_maps, core_ids=list(range(B)), trace=TRACE)
    LAST_EXEC_TIME_NS = res.exec_time_ns
    LAST_RESULT = res

    coords = np.empty((B, TH * TW, 4, 2), np.float32)
    weights = np.empty((B, TH * TW, 4), np.float32)
    density = np.empty((B, 1, H, W), np.float32)
    for i in range(B):
        r = res.results[i]
        coords[i] = r["coords"].reshape(TH * TW, 4, 2)
        w0 = r["w0"].reshape(-1)
        w1 = r["w1"].reshape(-1)
        w2 = r["w2"].reshape(-1)
        weights[i] = np.stack([w0, w1, w2, w2], axis=-1)
        density[i, 0] = r["density"]
    return coords, weights, density


# revision 22
# speedup vs baseline: 1.0760x; 1.0265x over previous
"""AdaptiveINR Trainium2 kernel (8-core data parallel).

Pipeline per image (one image per NeuronCore):
  contrast  = 0.5*sobel + 0.3*|lapl| + 0.2*gradm    (3x3 stencils on gray)
  density   = 0.1 + 0.9*sqrt(gauss5x5(minmax_norm(contrast)))
  coords    = base + (d>0.4)*off ;  weights = where(s<ns, d/ns, 0)
where d = density sampled at even rows/cols, off = input-independent
jax.random offsets (key 42) precomputed on host CPU.

Layout: 9 overlapping row-tiles per image, tile w holds image rows
122*w-3 .. 122*w+124 in partitions 0..127 (out-of-range rows zeroed).
Vertical convs (and the vertical half of the separable gauss, with the
horizontal taps accumulated in PSUM) are banded matmuls on TensorE;
horizontal shifts are shifted-AP ops on VectorE/GpSimd; transcendentals
on ScalarE.
"""

import functools
import os
import sys
import types

import numpy as np

B, H, W = 8, 1024, 1024
TH, TW = 512, 512
NT = 9          # row tiles per image
STRIDE = 122    # valid rows per tile
PAD = 3         # halo rows above the valid range

WR, WG, WB = 0.299, 0.587, 0.114
MIN_D, MAX_D = 0.1, 1.0
SPAN = MAX_D - MIN_D  # 0.9
LARGE = 1e30

TRACE = bool(os.environ.get("BASS_KERNEL_TRACE"))
LAST_EXEC_TIME_NS = None
LAST_RESULT = None


def _ensure_axon_hooks():
    """Register the NTFF profile hook (missing antenv.axon_hooks on this image)."""
    if "antenv.axon_hooks" in sys.modules:
        return
    try:
        import antenv  # noqa: F401

        mod = types.ModuleType("antenv.axon_hooks")
        _hook = {}
        mod.set_axon_ntff_profile_hook = lambda h: _hook.__setitem__("h", h)
        mod.get_axon_ntff_profile_hook = lambda: _hook.get("h")
        sys.modules["antenv.axon_hooks"] = mod
        from trn_agent_boot.trn_boot import _ntff_profile_via_ctypes

        mod.set_axon_ntff_profile_hook(
            _ntff_profile_via_ctypes("/opt/axon/libaxon_pjrt.so")
        )
    except Exception:
        pass


def _gauss1d():
    sigma = 5 / 6.0
    r = np.arange(5, dtype=np.float32) - 2
    k1 = np.exp(-0.5 * r**2 / sigma**2)
    k1 = (k1 / k1.sum()).astype(np.float32)
    return k1


@functools.lru_cache(maxsize=1)
def _host_consts():
    import jax
    import jax.numpy as jnp

    with jax.default_device(jax.devices("cpu")[0]):
        u = jax.random.uniform(jax.random.key(42), (B, TH, TW, 4, 2), jnp.float32)
        off = np.asarray((u - 0.5) * 0.8 * (2.0 / TH)).astype(np.float32)

    y = np.linspace(-1.0, 1.0, TH, dtype=np.float32)
    x = np.linspace(-1.0, 1.0, TW, dtype=np.float32)
    base = np.empty((TH, TW, 2), np.float32)
    base[:, :, 0] = y[:, None]
    base[:, :, 1] = x[None, :]
    p1b = (base[None, :, :, None, :] + off).astype(np.float32)  # [B, TH, TW, 4, 2]
    p1_all = np.ascontiguousarray(
        p1b.reshape(B, 128, 4, TW, 4, 2).reshape(B, 128, 4 * TW * 4 * 2))

    p0 = np.empty((128, 4, TW, 2), np.float32)
    p0[:, :, :, 0] = y.reshape(128, 4, 1)
    p0[:, :, :, 1] = x.reshape(1, 1, TW)
    p0 = np.ascontiguousarray(p0.reshape(128, 4 * TW * 2))

    # band matrices: band[k, m] = coeff of input partition k for output m
    def band(coeffs, zero_lo=0, zero_hi=0):
        bm = np.zeros((128, 128), np.float32)
        for dk, c in coeffs.items():
            for m in range(128):
                k = m + dk
                if 0 <= k < 128:
                    bm[k, m] = c
        if zero_lo:
            bm[:zero_lo, :] = 0.0
        if zero_hi:
            bm[-zero_hi:, :] = 0.0
        return bm

    k1 = _gauss1d()
    g0, g1, g2 = float(k1[0]), float(k1[1]), float(k1[2])
    band_v2 = band({-1: -1.0, 1: 1.0})
    band_2v2 = band({-1: -2.0, 1: 2.0})
    band_121 = band({-1: 1.0, 0: 2.0, 1: 1.0})
    band_l = band({-1: -1.0, 0: 4.0, 1: -1.0})
    gbands = []
    gb1 = {dk: float(k1[dk + 2]) * g1 for dk in range(-2, 3)}
    for scale in (g2 / g1, g0 / g1, 1.0):  # applied to nrm, h2, h1
        gb = {dk: v * scale for dk, v in gb1.items()}
        gbands.append([
            band(gb, zero_lo=PAD),           # tile 0
            band(gb),                        # middle
            band(gb, zero_hi=128 - 51),      # tile 8 (partitions >=51 are rows >=1024)
        ])

    # untrusted (non-duplicate, non-valid) rows for the min/max reduction
    maskl = np.zeros((128, NT), np.float32)
    maskl[0, :] = LARGE
    maskl[127, :] = LARGE
    maskl[1:3, 0] = LARGE
    maskl[51:, NT - 1] = LARGE

    allbands = np.ascontiguousarray(np.stack(
        [band_v2, band_2v2, band_121, band_l]
        + [gbands[gi][vi] for gi in range(3) for vi in range(3)], axis=0))
    return dict(
        p1_all=p1_all, p0=p0, maskl=np.ascontiguousarray(maskl),
        allbands=allbands, g0=g0, g1=g1, g2=g2,
    )


@functools.lru_cache(maxsize=1)
def _custom_ops():
    import concourse.dve_ops as dve_ops
    from concourse.dve_spec import Spec, Src0, Src1, C0, C1, C2, Zero, lower, sq, maxx
    from concourse.dve_uop import DveOpSpec

    def author(name, spec, rd1=True):
        if name in dve_ops._SUB_OPCODE_FOR_NAME:
            for op in dve_ops.OPS:
                if op.name == name:
                    return op
        row = max(dve_ops._SUB_OPCODE_FOR_NAME.values()) + 1
        dve_ops._SUB_OPCODE_FOR_NAME[name] = row
        shas = {}
        for ver in ("v3", "v4"):
            uops = lower(spec, ver=ver)
            shas[ver] = DveOpSpec(name=name, opcode=row, uops=uops, rd1_en=rd1).sha(ver)
        op = dve_ops.DveOp(name, spec, subdim=False, uops_sha=shas)
        dve_ops.OPS.append(op)
        dve_ops.CUSTOM_DVE_SPECS[name] = spec
        return op

    SQSUM = author("ANT_SQSUM", Spec(
        body=sq(Src0) + sq(Src1),
        reference=lambda i0, i1, s0, s1, c2: (
            i0.astype(np.float32) ** 2 + i1.astype(np.float32) ** 2
        ),
    ))
    body_abs = maxx(Src0 - Src1, Zero - (Src0 - Src1)) * C2
    ABSSUBS = author("ANT_ABSSUBS", Spec(
        body=body_abs,
        reference=lambda i0, i1, s0, s1, c2: (np.abs(
            i0.astype(np.float32) - i1.astype(np.float32)) * c2),
    ))
    ADD_ACCMAX = author("ANT_ADD_ACCMAX", Spec(
        body=Src0 + Src1, accum=maxx, accum_init=C0,
        reference=lambda i0, i1, s0, s1, c2: (
            (i0 + i1).astype(np.float32),
            np.maximum(s0, (i0 + i1).max(axis=-1, keepdims=True)).astype(np.float32),
        ),
    ))
    body_wq = Src0 - C2 * ((Src1 > C0) * Src0 + C2 * ((Src0 > C1) * Src0))
    WQ = author("ANT_WQ", Spec(
        body=body_wq,
        reference=lambda i0, i1, s0, s1, c2: (
            i0 - (i0 * c2) * (i0 > s0) - ((i1 * c2) * c2) * (i1 > s1)
        ).astype(np.float32),
    ))
    return dict(SQSUM=SQSUM, ABSSUBS=ABSSUBS, ADD_ACCMAX=ADD_ACCMAX, WQ=WQ)


@functools.lru_cache(maxsize=1)
def _build():
    import concourse.bass as bass
    import concourse.tile as tile
    from concourse import bacc, mybir
    from concourse.alu_op_type import AluOpType
    from concourse import bass_isa

    C = _host_consts()
    OPS = _custom_ops()
    f32 = mybir.dt.float32
    X = mybir.AxisListType.X
    AF = mybir.ActivationFunctionType

    nc = bacc.Bacc()

    img_e = nc.declare_dram_parameter("img", [3, H, W], f32, isOutput=False)
    p1_e = nc.declare_dram_parameter("p1", [128, 4 * TW * 4 * 2], f32, isOutput=False)
    p0_e = nc.declare_dram_parameter("p0", [128, 4 * TW * 2], f32, isOutput=False)
    maskl_e = nc.declare_dram_parameter("maskl", [128, NT], f32, isOutput=False)
    allbands_e = nc.declare_dram_parameter("allbands", [13, 128, 128], f32, isOutput=False)

    dens_e = nc.declare_dram_parameter("density", [H, W], f32, isOutput=True)
    w0_e = nc.declare_dram_parameter("w0", [128, 4 * TW], f32, isOutput=True)
    w1_e = nc.declare_dram_parameter("w1", [128, 4 * TW], f32, isOutput=True)
    w2_e = nc.declare_dram_parameter("w2", [128, 4 * TW], f32, isOutput=True)
    cout_e = nc.declare_dram_parameter("coords", [128, 4 * TW * 4 * 2], f32, isOutput=True)

    dcols_d = nc.dram_tensor("dcols_scratch", [H, TW], f32)

    with tile.TileContext(nc) as tc:
        with tc.tile_pool(name="persist", bufs=1) as pp:
            # constants (one packed load, issued off the critical sync queue)
            bandt = pp.tile([128, 13, 128], f32, tag="bandt")
            nc.scalar.dma_start(
                bandt[:], allbands_e[:].rearrange("n p j -> p n j"))
            bt = {nm: bandt[:, i, :] for i, nm in enumerate(
                ("band_v2", "band_2v2", "band_121", "band_l"))}
            gbt = [[bandt[:, 4 + gi * 3 + vi, :] for vi in range(3)]
                   for gi in range(3)]
            masklt = pp.tile([128, NT], f32, tag="masklt")
            nc.scalar.dma_start(masklt[:], maskl_e[:])
            cbias = pp.tile([128, 4], f32, tag="cbias")
            nc.vector.memset(cbias[:, 0:1], 2.5e-9)
            nc.vector.memset(cbias[:, 1:2], 4.0e-10)
            nc.vector.memset(cbias[:, 2:3], MIN_D)

            maxstack = pp.tile([128, NT], f32, tag="maxstack")
            minstack = pp.tile([128, NT], f32, tag="minstack")

            shp = tc.tile_pool(name="shp", bufs=1)
            shpool = shp.__enter__()
            CONT = shpool.tile([128, NT, W], f32, tag="CONT")
            scratch = shpool.tile([128, 4096], f32, tag="scratch")

            # ---------------- phase I: gray -> contrast ---------------------
            with tc.tile_pool(name="ph1", bufs=2) as wp, \
                 tc.tile_pool(name="io", bufs=3) as iop, \
                 tc.tile_pool(name="ps1", bufs=1, space="PSUM") as psp:
                for w in range(NT):
                    row0 = STRIDE * w - PAD
                    lo, hi = max(row0, 0), min(row0 + 128, H)
                    plo, phi = lo - row0, hi - row0

                    rgb = iop.tile([128, 3, W], f32, tag="rgb")
                    if plo > 0 or phi < 128:
                        nc.vector.memset(rgb[:], 0.0)
                    nc.sync.dma_start(
                        rgb[plo:phi, :, :],
                        img_e[:, lo:hi, :].rearrange("c r j -> r c j"))
                    R, G, Bc = rgb[:, 0, :], rgb[:, 1, :], rgb[:, 2, :]

                    t0a = wp.tile([128, W], f32, tag="t0a")
                    nc.scalar.mul(t0a[:], R, WR / WG)
                    t0 = wp.tile([128, W], f32, tag="t0")
                    nc.vector.tensor_tensor(t0[:], t0a[:], G, op=AluOpType.add)
                    g = wp.tile([128, W], f32, tag="g")
                    nc.vector.affine_then_add(g[:], Bc, t0[:], WB / WG, 0.0)

                    b = wp.tile([128, W], f32, tag="b")
                    nc.gpsimd.tensor_tensor(
                        b[:, 1:1023], g[:, 0:1022], g[:, 2:1024], op=AluOpType.add)
                    nc.scalar.copy(b[:, 0:1], g[:, 1:2])
                    nc.scalar.copy(b[:, 1023:1024], g[:, 1022:1023])

                    hd = wp.tile([128, W], f32, tag="hd")
                    nc.gpsimd.tensor_tensor(
                        hd[:, 1:1023], g[:, 2:1024], g[:, 0:1022],
                        op=AluOpType.subtract)
                    nc.scalar.copy(hd[:, 0:1], g[:, 1:2])
                    nc.scalar.mul(hd[:, 1023:1024], g[:, 1022:1023], -1.0)

                    v2_ps = psp.tile([128, W], f32, tag="v2_ps")
                    gy_ps = psp.tile([128, W], f32, tag="gy_ps")
                    gx_ps = psp.tile([128, W], f32, tag="gx_ps")
                    q1_ps = psp.tile([128, W], f32, tag="q1_ps")
                    for hh in (slice(0, 512), slice(512, 1024)):
                        nc.tensor.matmul(v2_ps[:, hh], bt["band_v2"], g[:, hh])
                        nc.tensor.matmul(gy_ps[:, hh], bt["band_2v2"], g[:, hh],
                                         start=True, stop=False)
                        nc.tensor.matmul(gy_ps[:, hh], bt["band_v2"], b[:, hh],
                                         start=False, stop=True)
                        nc.tensor.matmul(gx_ps[:, hh], bt["band_121"], hd[:, hh])
                        nc.tensor.matmul(q1_ps[:, hh], bt["band_l"], g[:, hh])

                    gxs = wp.tile([128, W], f32, tag="gxs")
                    nc.scalar.copy(gxs[:], gx_ps[:])

                    sob = wp.tile([128, W], f32, tag="sob")
                    nc.vector._custom_dve(OPS["SQSUM"], out=sob[:], in0=gxs[:], in1=gy_ps[:])
                    lapl = wp.tile([128, W], f32, tag="lapl")
                    nc.vector._custom_dve(
                        OPS["ABSSUBS"], out=lapl[:], in0=q1_ps[:], in1=b[:],
                        imm2=0.3 * WG)

                    grad = wp.tile([128, W], f32, tag="grad")
                    nc.vector._custom_dve(OPS["SQSUM"], out=grad[:], in0=hd[:], in1=v2_ps[:])
                    nc.vector._custom_dve(
                        OPS["SQSUM"], out=grad[:, 0:1], in0=hd[:, 1:2], in1=v2_ps[:, 0:1])
                    nc.vector._custom_dve(
                        OPS["SQSUM"], out=grad[:, 1023:1024], in0=hd[:, 1022:1023],
                        in1=v2_ps[:, 1023:1024])
                    if w == 0 or w == NT - 1:
                        p = PAD if w == 0 else (1023 - row0)
                        pa, pb = (p + 2, p) if w == 0 else (p, p - 2)
                        nc.gpsimd.dma_start(scratch[0:1, 0:1024], hd[p:p + 1, :])
                        nc.scalar.copy(scratch[0:1, 0:1], scratch[0:1, 1:2])
                        nc.scalar.copy(scratch[0:1, 1023:1024], scratch[0:1, 1022:1023])
                        nc.gpsimd.dma_start(scratch[0:1, 1024:2048], g[pa:pa + 1, :])
                        nc.gpsimd.dma_start(scratch[0:1, 3072:4096], g[pb:pb + 1, :])
                        nc.vector.tensor_tensor(
                            scratch[0:1, 1024:2048], scratch[0:1, 1024:2048],
                            scratch[0:1, 3072:4096], op=AluOpType.subtract)
                        nc.vector._custom_dve(
                            OPS["SQSUM"], out=scratch[0:1, 2048:3072],
                            in0=scratch[0:1, 0:1024], in1=scratch[0:1, 1024:2048])
                        nc.gpsimd.dma_start(grad[p:p + 1, :], scratch[0:1, 2048:3072])

                    sob_s = wp.tile([128, W], f32, tag="sob_s")
                    nc.scalar.activation(
                        sob_s[:], sob[:], AF.Sqrt,
                        bias=cbias[:, 0:1], scale=0.25 * WG * WG)
                    grad_s = wp.tile([128, W], f32, tag="grad_s")
                    nc.scalar.activation(
                        grad_s[:], grad[:], AF.Sqrt,
                        bias=cbias[:, 1:2], scale=0.04 * WG * WG)

                    t2 = wp.tile([128, W], f32, tag="t2")
                    nc.gpsimd.tensor_tensor(t2[:], sob_s[:], lapl[:], op=AluOpType.add)

                    nc.vector._custom_dve(
                        OPS["ADD_ACCMAX"], out=CONT[:, w, :], in0=t2[:], in1=grad_s[:],
                        s0=-LARGE, accum_out=maxstack[:, w:w + 1])
                    nc.vector.tensor_reduce(
                        minstack[:, w:w + 1], CONT[:, w, :], axis=X, op=AluOpType.min)

            # ---------------- min/max aggregation ---------------------------
            mm1 = pp.tile([128, NT], f32, tag="mm1")
            nc.vector.tensor_tensor(mm1[:], maxstack[:], masklt[:], op=AluOpType.subtract)
            mm2 = pp.tile([128, NT], f32, tag="mm2")
            nc.vector.tensor_tensor(mm2[:], minstack[:], masklt[:], op=AluOpType.add)
            mxr = pp.tile([128, 1], f32, tag="mxr")
            nc.vector.tensor_reduce(mxr[:], mm1[:], axis=X, op=AluOpType.max)
            mnr = pp.tile([128, 1], f32, tag="mnr")
            nc.vector.tensor_reduce(mnr[:], mm2[:], axis=X, op=AluOpType.min)
            negmn = pp.tile([128, 1], f32, tag="negmn")
            nc.vector.tensor_single_scalar(negmn[:], mnr[:], -1.0, op=AluOpType.mult)
            mxa = pp.tile([128, 1], f32, tag="mxa")
            nc.gpsimd.partition_all_reduce(mxa[:], mxr[:], channels=128,
                                           reduce_op=bass_isa.ReduceOp.max)
            nga = pp.tile([128, 1], f32, tag="nga")
            nc.gpsimd.partition_all_reduce(nga[:], negmn[:], channels=128,
                                           reduce_op=bass_isa.ReduceOp.max)
            rng = pp.tile([128, 1], f32, tag="rng")
            nc.vector.tensor_tensor(rng[:], mxa[:], nga[:], op=AluOpType.add)
            inv = pp.tile([128, 1], f32, tag="inv")
            nc.vector.reciprocal(inv[:], rng[:])
            mnv = pp.tile([128, 1], f32, tag="mnv")
            nc.vector.tensor_single_scalar(mnv[:], nga[:], -1.0, op=AluOpType.mult)

            # ---------------- phase II: normalize + gauss + density ---------
            with tc.tile_pool(name="ph2", bufs=2) as wp2, \
                 tc.tile_pool(name="ps2", bufs=2, space="PSUM") as ps2:
                for w in range(NT):
                    vi = 0 if w == 0 else (2 if w == NT - 1 else 1)
                    nrm = wp2.tile([128, W], f32, tag="nrm")
                    nc.vector.tensor_scalar(
                        nrm[:], CONT[:, w, :], mnv[:], inv[:],
                        op0=AluOpType.subtract, op1=AluOpType.mult)
                    h1 = wp2.tile([128, W], f32, tag="h1")
                    nc.gpsimd.tensor_tensor(
                        h1[:, 1:1023], nrm[:, 0:1022], nrm[:, 2:1024], op=AluOpType.add)
                    nc.scalar.copy(h1[:, 0:1], nrm[:, 1:2])
                    nc.scalar.copy(h1[:, 1023:1024], nrm[:, 1022:1023])
                    h2 = wp2.tile([128, W], f32, tag="h2")
                    nc.gpsimd.tensor_tensor(
                        h2[:, 2:1022], nrm[:, 0:1020], nrm[:, 4:1024], op=AluOpType.add)
                    nc.scalar.copy(h2[:, 0:2], nrm[:, 2:4])
                    nc.scalar.copy(h2[:, 1022:1024], nrm[:, 1020:1022])

                    x_ps = ps2.tile([128, W], f32, tag="x_ps")
                    for hh in (slice(0, 512), slice(512, 1024)):
                        nc.tensor.matmul(x_ps[:, hh], gbt[0][vi], nrm[:, hh],
                                         start=True, stop=False)
                        nc.tensor.matmul(x_ps[:, hh], gbt[1][vi], h2[:, hh],
                                         start=False, stop=False)
                        nc.tensor.matmul(x_ps[:, hh], gbt[2][vi], h1[:, hh],
                                         start=False, stop=True)

                    r = wp2.tile([128, W], f32, tag="r")
                    nc.scalar.activation(r[:], x_ps[:], AF.Relu, scale=SPAN * SPAN)
                    qq = wp2.tile([128, W], f32, tag="qq")
                    nc.scalar.activation(qq[:], r[:], AF.Sqrt)
                    dens = wp2.tile([128, W], f32, tag="dens")
                    nc.scalar.activation(dens[:], qq[:], AF.Identity, bias=cbias[:, 2:3])
                    dcl = wp2.tile([128, TW], f32, tag="dcl")
                    nc.scalar.copy(dcl[:], dens[:, 0:1024:2])
                    lo_v = STRIDE * w
                    hi_v = min(lo_v + STRIDE, H)
                    pv0, pv1 = PAD, PAD + (hi_v - lo_v)
                    eng = (nc.scalar, nc.gpsimd)[w % 2]
                    eng.dma_start(dens_e[lo_v:hi_v, :], dens[pv0:pv1, :])
                    eng2 = (nc.gpsimd, nc.scalar)[w % 2]
                    eng2.dma_start(dcols_d[lo_v:hi_v, :], dcl[pv0:pv1, :])

            shp.__exit__(None, None, None)

            # ---------------- sample stage: weights + coords ----------------
            with tc.tile_pool(name="samp", bufs=1) as sp, \
                 tc.tile_pool(name="cq", bufs=1) as cqp:
                cqt = sp.tile([128, 4 * TW * 4 * 2], f32, tag="cqt")
                for qi in range(4):
                    nc.sync.dma_start(cqt[:, qi * 4096:(qi + 1) * 4096],
                                      p1_e[:, qi * 4096:(qi + 1) * 4096])
                D = sp.tile([128, 4, TW], f32, tag="D")
                nc.sync.dma_start(
                    D[:], dcols_d[:].rearrange("(q s) j -> q s j", s=8)[:, 0:8:2, :])
                Dv = D[:].rearrange("p s j -> p (s j)")
                m2 = sp.tile([128, 4 * TW], f32, tag="m2")
                nc.vector.tensor_single_scalar(m2[:], Dv, 0.4, op=AluOpType.is_gt)
                m2i = sp.tile([128, 4 * TW], mybir.dt.uint8, tag="m2i")
                nc.vector.tensor_single_scalar(m2i[:], Dv, 0.4, op=AluOpType.is_le)
                m4 = sp.tile([128, 4 * TW], f32, tag="m4")
                nc.vector.tensor_single_scalar(m4[:], Dv, 0.7, op=AluOpType.is_gt)
                acc = sp.tile([128, 4 * TW], f32, tag="acc")
                nc.vector.tensor_scalar(
                    acc[:], m4[:], -0.25, 1.0, op0=AluOpType.mult, op1=AluOpType.add)
                nc.vector.scalar_tensor_tensor(
                    acc[:], m2[:], -0.5, acc[:],
                    op0=AluOpType.mult, op1=AluOpType.add)
                wq = sp.tile([128, 4 * TW], f32, tag="wq")
                nc.vector.tensor_tensor(wq[:], Dv, acc[:], op=AluOpType.mult)
                w1t = sp.tile([128, 4 * TW], f32, tag="w1t")
                nc.vector.tensor_tensor(w1t[:], wq[:], m2[:], op=AluOpType.mult)
                w2t = sp.tile([128, 4 * TW], f32, tag="w2t")
                nc.vector.tensor_tensor(w2t[:], wq[:], m4[:], op=AluOpType.mult)
                nc.scalar.dma_start(w0_e[:], wq[:])
                nc.scalar.dma_start(w1_e[:], w1t[:])
                nc.gpsimd.dma_start(w2_e[:], w2t[:])

                p0t = sp.tile([128, 4 * TW * 2], f32, tag="p0t")
                nc.sync.dma_start(p0t[:], p0_e[:])
                m2e8 = sp.tile([128, 4 * TW, 8], mybir.dt.uint8, tag="m2e8")
                nc.gpsimd.tensor_copy(
                    m2e8[:], m2i[:].rearrange(
                        "p (n o) -> p n o", o=1).broadcast_to((128, 4 * TW, 8)))
                # cqt is prefilled with p1 = base+off; overwrite base where ns==1
                for hf in range(2):
                    p0eh = cqp.tile([128, 2 * TW * 4 * 2], f32, tag="p0eh")
                    nc.scalar.copy(
                        p0eh[:].rearrange("p (n s c) -> p n s c", s=4, c=2),
                        p0t[:, hf * 2048:(hf + 1) * 2048].rearrange(
                            "p (n o c) -> p n o c", o=1, c=2)
                        .broadcast_to((128, 2 * TW, 4, 2)))
                    nc.vector.copy_predicated(
                        cqt[:, hf * 8192:(hf + 1) * 8192],
                        m2e8[:, hf * 2 * TW:(hf + 1) * 2 * TW, :].rearrange(
                            "p n o -> p (n o)"),
                        p0eh[:])
                    for r_ in (2 * hf, 2 * hf + 1):
                        eng = (nc.sync, nc.scalar, nc.gpsimd, nc.sync)[r_]
                        eng.dma_start(
                            cout_e[:, r_ * 4096:(r_ + 1) * 4096],
                            cqt[:, r_ * 4096:(r_ + 1) * 4096])

    nc.finalize()
    return nc


def kernel(img, target_height, target_width, **_kw):
    global LAST_EXEC_TIME_NS, LAST_RESULT
    th, tw = int(target_height), int(target_width)
    img = np.ascontiguousarray(np.asarray(img, dtype=np.float32))
    assert img.shape == (B, 3, H, W) and th == TH and tw == TW, (
        f"kernel hardcoded for img(8,3,1024,1024), th=tw=512; got {img.shape} {th} {tw}")

    if TRACE:
        _ensure_axon_hooks()

    C = _host_consts()
    nc = _build()

    from concourse.bass_utils import run_bass_kernel_spmd

    in_maps = []
    for i in range(B):
        m = {
            "img": img[i],
            "p1": C["p1_all"][i],
            "p0": C["p0"],
            "maskl": C["maskl"],
            "allbands": C["allbands"],
        }
        in_maps.append(m)
    res = run_bass_kernel_spmd(
        nc, in_maps, core_ids=list(range(B)), trace=TRACE)
    LAST_EXEC_TIME_NS = res.exec_time_ns
    LAST_RESULT = res

    coords = np.empty((B, TH * TW, 4, 2), np.float32)
    weights = np.empty((B, TH * TW, 4), np.float32)
    density = np.empty((B, 1, H, W), np.float32)
    for i in range(B):
        r = res.results[i]
        coords[i] = r["coords"].reshape(TH * TW, 4, 2)
        w0 = r["w0"].reshape(-1)
        w1 = r["w1"].reshape(-1)
        w2 = r["w2"].reshape(-1)
        weights[i] = np.stack([w0, w1, w2, w2], axis=-1)
        density[i, 0] = r["density"]
    return coords, weights, density
